# revision 43
# baseline (speedup 1.0000x reference)
"""Trainium2 8-core kernel for nn_Attention_70892730187933 (sparse multi-query attention).

Sharding: sequence-parallel over query rows. Core c owns rows {i : i % 8 == c},
as 2 blocks of 128 rows (block0 < 1024, block1 >= 1024). Causal trimming:
block0 needs key j-tiles 0..8, block1 needs 0..16 (key space padded to
17*128 = 2176 incl. 2 null cols). No collectives; host concatenates rows.

Activations stay transposed [feature, token] so every matmul contraction dim
lands on partitions with no on-device activation transposes. LayerNorm stats
come from PE ones-column matmuls on bf16 x^T (raw moments); the mean
correction folds into the Q projection (rank-1 srow term). The bias+mask is
applied MULTIPLICATIVELY: host precomputes exp(bias+mask) (masked -> 0), and
after ScalarE computes exp(sim) from PSUM, DVE multiplies it in bf16 -- no
per-tile identity bias matmuls on the Tensor
engine. PE stationary loads (LDWEIGHTS ~145ns each) are halved where tiles
repeat: stats/broadcast ones vectors, kv projection run ct-outer (one wkv
load serves 4 column chunks), and o-matmuls process TWO head-pairs jointly
(one V-tile load serves both pairs' E) via ldweights=False on the repeats.
Rowsums ride as a ones-column in V; per-pair SBUF->SBUF DMAs scatter them
into a [64, 64] tile laid out so a direct flatten DMA yields the even/odd-
head-permuted reciprocal row for 8 PE broadcast matmuls + 3 normalize mults.

Raw Block + explicit semaphores: this walrus build rejects instructions with
multiple attached sync waits, so Tile-generated sync cannot compile; every
cross-engine wait here is its own sequencer instruction. The builder plans all
five engine programs in one pass (semaphore counters known at plan time), then
emits them inside one Block.
"""

import sys
import numpy as np

sys.path.insert(0, "/opt/trn_rl_repo")

B, N, DIM, HEADS, DIM_HEAD, NUM_NULL = 1, 2048, 1024, 16, 64, 2
INNER = HEADS * DIM_HEAD
EPS = 1e-5
NCORES = 8
JT = 17
JPAD = JT * 128
NQ = 256
MASK_VAL = -30000.0
SH_JT = 9
NPAIR = HEADS // 2
EWP = SH_JT * 512 + (JT - SH_JT) * 256  # 6656: packed pair E/ebias width
# psum chunks: 5 per pair so two [65,512] o-accumulators fit alongside.
# A-region: ck0,ck2,ck4 (max 1536); B-region: ck1,ck3 (max 1536).
CH_OF_JT = [0, 0, 0, 1, 1, 1, 2, 2, 2, 3, 3, 3, 3, 4, 4, 4, 4]
CH_LEN = [1536, 1536, 1536, 1024, 1024]
CH_ECOL = [0, 1536, 3072, 4608, 5632]
CH_ENG = ["vector", "vector", "vector", "vector", "vector"]  # E-mult engine
NCH = 5

_CACHE = {}


def _ecol(jt):
    """Column of j-tile jt in packed pair E/ebias layout."""
    return jt * 512 if jt < SH_JT else SH_JT * 512 + (jt - SH_JT) * 256


def _ccol(jt):
    """Column of j-tile jt inside its psum chunk."""
    if jt < SH_JT:
        return (jt % 3) * 512
    if jt < 13:
        return (jt - 9) * 256
    return (jt - 13) * 256


def _build_graph():
    from contextlib import ExitStack
    import concourse.bass as bass
    import concourse.mybir as mybir

    dt = mybir.dt
    F32, BF16 = dt.float32, dt.bfloat16
    AF = mybir.ActivationFunctionType
    OP = mybir.AluOpType
    nc = bass.Bass()

    d_in = {}
    for name, shape, ty in [
        ("xtq", [DIM, NQ], BF16), ("xt", [DIM, N], BF16),
        ("wq", [DIM, INNER], BF16), ("srow", [1, INNER], BF16),
        ("wkv", [DIM, 2 * DIM_HEAD], BF16), ("nkvt", [128, NUM_NULL], BF16),
        ("wout", [INNER, DIM], BF16), ("ibf", [128, 128], BF16),
        ("onesbf", [1, 128], BF16), ("onesf", [1, 128], F32),
        ("ebias", [NPAIR, 128, EWP], BF16),
    ]:
        d_in[name] = nc.declare_dram_parameter(name, shape, ty, isOutput=False)
    out_d = nc.declare_dram_parameter("out", [DIM, NQ], F32, isOutput=True)

    ctx = ExitStack()
    sb = {}
    for name, shape, ty in [
        ("ibf", [128, 128], BF16), ("onesbf", [1, 128], BF16),
        ("onesf", [1, 128], F32), ("srow", [1, INNER], BF16),
        ("zb", [128, 1], F32), ("epsb", [128, 1], F32),
        ("onescol", [128, 1], BF16),
        ("wq", [128, 8 * INNER], BF16), ("wout", [128, 8 * DIM], BF16),
        ("wkv", [128, 8 * 128], BF16), ("xt", [128, 8 * N], BF16),
        ("xtq", [128, 8 * NQ], BF16), ("xsq", [128, 8 * NQ], BF16),
        ("lnrow", [1, 1024], F32),  # 0:256 negmu, 256:512 ex2, 512:768 var, 768:1024 rsq
        ("negmurs", [1, NQ], BF16),
        ("xst", [128, 8 * NQ], BF16), ("qtmp", [128, 2 * NQ], BF16),
        ("kv", [128, JPAD], BF16), ("vsb", [128, JT * 65], BF16),
        ("e0", [128, EWP], BF16), ("e1", [128, EWP], BF16),
        ("b0", [128, EWP], BF16), ("b1", [128, EWP], BF16),
        ("oT", [64, HEADS * NQ], BF16), ("rs_row", [65, 2 * 512], F32),
        ("rs64", [64, 64], F32), ("r64f", [64, 64], F32), ("r64b", [64, 64], BF16),
        ("recipflat", [1, HEADS * NQ], BF16),
        ("oTn", [128, 8 * NQ], BF16), ("oTn_lo", [64, 8 * NQ], BF16),
        ("outsb", [128, 4 * NQ], F32),
    ] + [(f"qh{h}", [64, 2 * NQ], BF16) for h in range(NPAIR)]:
        sb[name] = ctx.enter_context(nc.sbuf_tensor("sb_" + name, shape, ty))

    qh = [sb[f"qh{h}"] for h in range(NPAIR)]
    esb = [sb["e0"], sb["e1"]]
    bsb = [sb["b0"], sb["b1"]]

    # PSUM: early tensors freed before head-loop tensors are allocated.
    early = ExitStack()
    kvp = early.enter_context(nc.psum_tensor("kvp", [128, 1024], F32))
    qp = [early.enter_context(nc.psum_tensor(f"qp{i}", [128, NQ], F32))
          for i in range(2)]
    vp = [early.enter_context(nc.psum_tensor(f"vp{i}", [128, 64], BF16))
          for i in range(2)]
    stp2 = early.enter_context(nc.psum_tensor("stp2", [1, 512], F32))
    rbp = early.enter_context(nc.psum_tensor("rbp", [128, NQ], F32))
    early.close()
    simA = ctx.enter_context(nc.psum_tensor("simA", [128, 1536], F32))
    simB = ctx.enter_context(nc.psum_tensor("simB", [128, 1536], F32))
    opp2 = ctx.enter_context(nc.psum_tensor("opp2", [65, 1024], F32))

    # ------- planner -------
    plan = {"sync": [], "tensor": [], "vector": [], "scalar": [], "gpsimd": []}
    DSEMS = ("dk", "dxq", "dqx", "dsm", "dw",
             "db0", "db1", "dq", "dr", "dm", "dn", "do")
    cnt = {"p": 0, "v": 0, "s": 0, "g": 0, **{k: 0 for k in DSEMS}}
    SEM = {}
    ESEM = {"vector": "v", "gpsimd": "g", "scalar": "s"}

    def wait(eng, sem, thr):
        if thr > 0:
            plan[eng].append(lambda e, s=sem, t=thr: e.wait_ge(SEM[s], t))

    def dma(sem, out, in_, eng="sync"):
        cnt[sem] += 16
        plan[eng].append(
            lambda e, s=sem, o=out, i=in_: e.dma_start(out=o, in_=i)
            .then_inc(SEM[s], 16))
        return cnt[sem]

    def inc(eng, sem, fn):
        # DVE/ACT/Pool pipelines expose writes only after a drain; put the sem
        # update on the drain so consumers see committed data.
        cnt[sem] += 1
        if eng in ("vector", "scalar", "gpsimd"):
            plan[eng].append(lambda e, f=fn: f(e))
            plan[eng].append(lambda e, s=sem: e.drain().then_inc(SEM[s], 1))
        else:
            plan[eng].append(lambda e, f=fn, s=sem: f(e).then_inc(SEM[s], 1))
        return cnt[sem]

    def run(eng, fn):
        plan[eng].append(fn)

    def runD(eng, fn):
        # run + drain, for same-engine RAW chains without a semaphore inc
        plan[eng].append(fn)
        plan[eng].append(lambda e: e.drain())

    def mm_reuse(e, *args, **kwargs):
        # matmul whose stationary was loaded by the immediately preceding
        # matmul on PE: suppress this instruction's LDWEIGHTS.
        m = e.matmul(*args, **kwargs)
        m.ins.ldweights = False
        return m

    # ========== DMA issue order (sync queue = priority order) ==========
    def dma8(sem, nm, eng="sync"):
        return dma(sem, sb[nm][:].rearrange("p (c f) -> p c f", c=8),
                   d_in[nm][:].rearrange("(c p) f -> p c f", c=8), eng=eng)

    d_xq = dma8("dxq", "xtq")
    dma8("dqx", "wq")
    d_qx = cnt["dqx"]
    for nm in ("ibf", "onesbf", "onesf", "srow"):
        dma("dsm", sb[nm][:], d_in[nm][:])
    d_sm0 = dma("dsm", sb["kv"][:, 0:NUM_NULL], d_in["nkvt"][:])
    dma8("dk", "wkv")
    dma8("dk", "xt")
    d_kv = cnt["dk"]
    for p in range(2):
        dma(f"db{p}", bsb[p][:], d_in["ebias"][p])
    d_w = dma8("dw", "wout")  # after ebias: not needed until the tail

    # ========== VECTOR: memsets ==========
    run("vector", lambda e: e.memset(sb["zb"][:], 0.0))
    run("vector", lambda e: e.memset(sb["epsb"][:], EPS))
    run("vector", lambda e: e.memset(sb["onescol"][:], 1.0))
    run("vector", lambda e: e.memset(sb["vsb"][:], 1.0))
    v_memset = inc("vector", "v",
                   lambda e: e.memset(sb["kv"][:, NUM_NULL + N:JPAD], 0.0))

    # ========== LN stats (row layout, raw moments) ==========
    wait("scalar", "dxq", d_xq)
    wait("scalar", "v", v_memset)  # zb ready
    s_xsq = inc("scalar", "s", lambda e: e.activation(
        out=sb["xsq"][:], in_=sb["xtq"][:], func=AF.Square, bias=sb["zb"][:]))

    # tensor: musum (cols 0:256) and sqsum (cols 256:512) into stp2;
    # all 16 matmuls share the onescol stationary (one LDWEIGHTS).
    wait("tensor", "dxq", d_xq)
    wait("tensor", "v", v_memset)  # onescol ready
    for ct in range(8):
        def fn(e, ct=ct):
            f = e.matmul if ct == 0 else (lambda *a, **k: mm_reuse(e, *a, **k))
            return f(stp2[0:1, 0:NQ], sb["onescol"][:],
                     sb["xtq"][:, ct * NQ:(ct + 1) * NQ],
                     start=(ct == 0), stop=(ct == 7))
        if ct == 7:
            p_mu = inc("tensor", "p", fn)
        else:
            run("tensor", fn)

    wait("tensor", "s", s_xsq)
    for ct in range(8):
        fn = lambda e, ct=ct: mm_reuse(
            e, stp2[0:1, NQ:2 * NQ], sb["onescol"][:],
            sb["xsq"][:, ct * NQ:(ct + 1) * NQ],
            start=(ct == 0), stop=(ct == 7))
        if ct == 7:
            p_sq = inc("tensor", "p", fn)
        else:
            run("tensor", fn)

    # vector rowops (RAW chains -> drained)
    neg_mu = sb["lnrow"][0:1, 0:256]
    ex2 = sb["lnrow"][0:1, 256:512]
    var_r = sb["lnrow"][0:1, 512:768]
    rsq_r = sb["lnrow"][0:1, 768:1024]
    wait("vector", "p", p_mu)
    runD("vector", lambda e: e.tensor_scalar_mul(
        out=neg_mu, in0=stp2[0:1, 0:NQ], scalar1=-1.0 / DIM))
    wait("vector", "p", p_sq)
    runD("vector", lambda e: e.tensor_scalar_mul(
        out=ex2, in0=stp2[0:1, NQ:2 * NQ], scalar1=1.0 / DIM))
    runD("vector", lambda e: e.tensor_tensor(
        out=var_r, in0=neg_mu, in1=neg_mu, op=OP.mult))
    v_var = inc("vector", "v", lambda e: e.tensor_tensor(
        out=var_r, in0=ex2, in1=var_r, op=OP.subtract))
    # scalar: rsq = exp(-0.5 * ln(var + eps))
    wait("scalar", "v", v_var)
    runD("scalar", lambda e: e.activation(
        out=var_r, in_=var_r, func=AF.Ln, scale=1.0, bias=sb["epsb"][0:1, :]))
    s_rsq = inc("scalar", "s", lambda e: e.activation(
        out=rsq_r, in_=var_r, func=AF.Exp, scale=-0.5, bias=sb["zb"][0:1, :]))
    wait("vector", "s", s_rsq)
    v_nmr = inc("vector", "v", lambda e: e.tensor_tensor(
        out=sb["negmurs"][:], in0=neg_mu, in1=rsq_r, op=OP.mult))

    # tensor: rsqb broadcast [128, 256] (f32 matmul)
    wait("tensor", "dsm", d_sm0)  # onesf (+ibf/srow/nkvt) loaded
    wait("tensor", "s", s_rsq)
    p_rsqb = inc("tensor", "p", lambda e: e.matmul(
        rbp[:], sb["onesf"][0:1, :], rsq_r, start=True, stop=True))

    # vector: xst = xtq * rsqb
    wait("vector", "p", p_rsqb)
    for ct in range(8):
        fn = lambda e, ct=ct: e.tensor_tensor(
            out=sb["xst"][:, ct * NQ:(ct + 1) * NQ],
            in0=sb["xtq"][:, ct * NQ:(ct + 1) * NQ], in1=rbp[:], op=OP.mult)
        if ct == 7:
            v_xst = inc("vector", "v", fn)
        else:
            run("vector", fn)

    # ========== TENSOR: q projection (qp double-buffered) ==========
    wait("tensor", "v", v_xst)
    wait("tensor", "dqx", d_qx)
    wait("tensor", "v", v_nmr)
    p_q = [0] * 8
    v_qtmp = [0] * 8
    for dtile in range(8):
        pb = qp[dtile % 2]
        if dtile >= 2:
            wait("tensor", "v", v_qtmp[dtile - 2])
        for ct in range(8):
            run("tensor", lambda e, pb=pb, dtile=dtile, ct=ct: e.matmul(
                pb[:],
                sb["wq"][:, ct * INNER + dtile * 128:ct * INNER + (dtile + 1) * 128],
                sb["xst"][:, ct * NQ:(ct + 1) * NQ],
                start=(ct == 0), stop=False))
        p_q[dtile] = inc("tensor", "p", lambda e, pb=pb, dtile=dtile: e.matmul(
            pb[:], sb["srow"][0:1, dtile * 128:(dtile + 1) * 128],
            sb["negmurs"][:], start=False, stop=True))
        wait("vector", "p", p_q[dtile])
        run("vector", lambda e, pb=pb, dtile=dtile: e.tensor_copy(
            qh[dtile][0:64, :].rearrange("a (b h c) -> a b h c", b=2, h=2)[:, :, 0, :],
            pb[0:64, :].rearrange("a (b c) -> a b c", b=2)))
        slot = dtile % 2
        if dtile >= 2:
            wait("vector", "dq", 16 * dtile)  # qtmp slot reuse (all issued)
        v_qtmp[dtile] = inc("vector", "v", lambda e, pb=pb, slot=slot:
                            e.tensor_copy(
                                sb["qtmp"][64:128, slot * NQ:(slot + 1) * NQ],
                                pb[64:128, :]))
        # small latency-critical DMA: issue on the (idle) gpsimd queue so it
        # bypasses the big weight/ebias streams queued on the sync queue
        wait("gpsimd", "v", v_qtmp[dtile])
        dma("dq",
            qh[dtile][0:64, :].rearrange("a (b h c) -> a b h c", b=2, h=2)[:, :, 1, :],
            sb["qtmp"][64:128, slot * NQ:(slot + 1) * NQ]
            .rearrange("a (b c) -> a b c", b=2), eng="gpsimd")

    # ========== TENSOR: kv matmuls, ct-outer (one wkv load per ct) ==========
    wait("tensor", "dk", d_kv)
    s_kvevac = [0] * 4
    p_kvh = [0, 0]
    for half in range(2):
        if half == 1:
            wait("tensor", "s", s_kvevac[1])  # kvp reuse
        for ct in range(8):
            for chh in range(2):
                ch = half * 2 + chh
                def fn(e, ct=ct, ch=ch, chh=chh):
                    f = e.matmul if chh == 0 else (
                        lambda *a, **k: mm_reuse(e, *a, **k))
                    return f(
                        kvp[:, chh * 512:(chh + 1) * 512],
                        sb["wkv"][:, ct * 128:(ct + 1) * 128],
                        sb["xt"][:, ct * N + ch * 512:ct * N + (ch + 1) * 512],
                        start=(ct == 0), stop=(ct == 7))
                if ct == 7 and chh == 1:
                    p_kvh[half] = inc("tensor", "p", fn)
                else:
                    run("tensor", fn)
        wait("scalar", "p", p_kvh[half])
        for chh in range(2):
            ch = half * 2 + chh
            s_kvevac[ch] = inc("scalar", "s", lambda e, ch=ch, chh=chh:
                               e.activation(
                                   out=sb["kv"][:, NUM_NULL + ch * 512:
                                                NUM_NULL + (ch + 1) * 512],
                                   in_=kvp[:, chh * 512:(chh + 1) * 512],
                                   func=AF.Copy))

    # ========== TENSOR: v transposes (vp double-buffered) ==========
    p_vt = [0] * JT
    v_vcopy = [0] * JT
    for jt in range(JT):
        pb = vp[jt % 2]
        ch_hi = min(3, ((jt + 1) * 128 - 1 - NUM_NULL) // 512)
        wait("tensor", "s", s_kvevac[ch_hi])
        if jt == JT - 1:
            wait("tensor", "v", v_memset)
        if jt >= 2:
            wait("tensor", "v", v_vcopy[jt - 2])
        p_vt[jt] = inc("tensor", "p", lambda e, pb=pb, jt=jt: e.transpose(
            pb[:], sb["kv"][64:128, jt * 128:(jt + 1) * 128],
            sb["ibf"][64:128, 64:128]))
        wait("vector", "p", p_vt[jt])
        v_vcopy[jt] = inc("vector", "v", lambda e, pb=pb, jt=jt: e.tensor_copy(
            sb["vsb"][:, jt * 65:jt * 65 + 64], pb[:]))
    v_vsb = v_vcopy[JT - 1]

    # ========== PAIR LOOP: sims per pair, o-matmuls joint per pair-pair ====
    v_pre_heads = cnt["v"]
    s_exp = [[0] * NCH for _ in range(NPAIR)]
    e_mult = [[None] * NCH for _ in range(NPAIR)]  # (sem_name, count)
    p_simc = [[0] * NCH for _ in range(NPAIR)]
    p_odone = [0] * (NPAIR // 2)
    v_oevac = [0] * (NPAIR // 2)

    wait("tensor", "s", s_kvevac[3])
    wait("tensor", "v", v_pre_heads)  # early-psum reuse guard
    wait("tensor", "dq", 16 * 8)      # all odd-half q DMAs done

    def emit_pair_sims(p):
        """sims + exp + E-mult for pair p (5 chunks)."""
        eh = esb[p % 2]
        bh = bsb[p % 2]
        qpair = qh[p][0:64, :]
        qsolo = qh[p][0:64, NQ:2 * NQ]
        for ck in range(NCH):
            ps = simA if ck in (0, 2, 4) else simB
            jts = [jt for jt in range(JT) if CH_OF_JT[jt] == ck]
            # psum region reuse: wait on exp of the previous occupant
            if ck in (0, 1):
                if p >= 1:
                    wait("tensor", "s", s_exp[p - 1][{0: 4, 1: 3}[ck]])
            else:
                wait("tensor", "s", s_exp[p][ck - 2])
            if ck != 4:
                for jt in jts:
                    w = 512 if jt < SH_JT else 256
                    rhs = qpair if jt < SH_JT else qsolo
                    fn = lambda e, ps=ps, jt=jt, r=rhs, w=w: e.matmul(
                        ps[:, _ccol(jt):_ccol(jt) + w],
                        sb["kv"][0:64, jt * 128:(jt + 1) * 128], r,
                        start=True, stop=True)
                    if jt == jts[-1]:
                        p_simc[p][ck] = inc("tensor", "p", fn)
                    else:
                        run("tensor", fn)
            else:
                # last chunk: additive bias via identity matmuls on PE (host
                # packs RAW bias+mask here, not exp) -- exp then yields final
                # E with no DVE hop on the joint-o critical path. Group j-
                # tiles bank-pairwise: only one open accumulation group per
                # psum bank is allowed.
                wait("tensor", f"db{p % 2}", 16 * (p // 2 + 1))
                for grp in ((13, 15), (14, 16)):
                    for jt in grp:
                        run("tensor", lambda e, ps=ps, jt=jt: e.matmul(
                            ps[:, _ccol(jt):_ccol(jt) + 256],
                            sb["kv"][0:64, jt * 128:(jt + 1) * 128], qsolo,
                            start=True, stop=False))
                    for ji, jt in enumerate(grp):
                        def fnb(e, jt=jt, ji=ji, ps=ps):
                            f = e.matmul if ji == 0 else (
                                lambda *a, **k: mm_reuse(e, *a, **k))
                            return f(ps[:, _ccol(jt):_ccol(jt) + 256],
                                     sb["ibf"][:],
                                     bh[:, _ecol(jt):_ecol(jt) + 256],
                                     start=False, stop=True)
                        if grp == (14, 16) and ji == 1:
                            p_simc[p][ck] = inc("tensor", "p", fnb)
                        else:
                            run("tensor", fnb)
            # SCALAR: exp for this chunk
            wait("scalar", "p", p_simc[p][ck])
            if ck == 0 and p >= 2:
                wait("scalar", "p", p_odone[p // 2 - 1])  # E slot reuse
            ln = CH_LEN[ck]
            s_exp[p][ck] = inc("scalar", "s", lambda e, ps=ps, ck=ck, ln=ln,
                               eh=eh: e.activation(
                                   out=eh[:, CH_ECOL[ck]:CH_ECOL[ck] + ln],
                                   in_=ps[:, 0:ln], func=AF.Exp, bias=sb["zb"][:]))
            if ck == 4:
                # bias already added in PSUM; exp output IS the final E
                e_mult[p][ck] = ("s", s_exp[p][ck])
                continue
            # DVE: E *= exp(bias) in place (bf16, all-SBUF)
            eng = CH_ENG[ck]
            sem = ESEM[eng]
            if ck in (0, 1):  # first chunk on each engine: ebias slot loaded
                wait(eng, f"db{p % 2}", 16 * (p // 2 + 1))
            wait(eng, "s", s_exp[p][ck])
            n = inc(eng, sem, lambda e, ck=ck, ln=ln, eh=eh, bh=bh:
                    e.tensor_tensor(
                        out=eh[:, CH_ECOL[ck]:CH_ECOL[ck] + ln],
                        in0=eh[:, CH_ECOL[ck]:CH_ECOL[ck] + ln],
                        in1=bh[:, CH_ECOL[ck]:CH_ECOL[ck] + ln],
                        op=OP.mult))
            e_mult[p][ck] = (sem, n)

    for pp in range(NPAIR // 2):
        pa, pb_ = 2 * pp, 2 * pp + 1
        emit_pair_sims(pa)
        emit_pair_sims(pb_)

        # TENSOR: joint o-matmuls for pairs (pa, pb_): per jt one V-tile
        # load serves both pairs (second matmul reuses the stationary).
        if pp == 0:
            wait("tensor", "v", v_vsb)
        if pp >= 1:
            wait("tensor", "v", v_oevac[pp - 1])  # opp2 reuse
        ea, eb_ = esb[0], esb[1]
        for jt in range(JT):
            ck = CH_OF_JT[jt]
            if jt == 0 or _ccol(jt) == 0:
                for pq in (pa, pb_):
                    sem, n = e_mult[pq][ck]
                    wait("tensor", sem, n)
            w, eoff = (512, 0) if jt < SH_JT else (256, 256)
            fn_a = lambda e, jt=jt, w=w, eoff=eoff: e.matmul(
                opp2[0:65, eoff:eoff + w],
                sb["vsb"][:, jt * 65:jt * 65 + 65],
                ea[:, _ecol(jt):_ecol(jt) + w],
                start=(jt == 0), stop=(jt == JT - 1))
            fn_b = lambda e, jt=jt, w=w, eoff=eoff: mm_reuse(
                e, opp2[0:65, 512 + eoff:512 + eoff + w],
                sb["vsb"][:, jt * 65:jt * 65 + 65],
                eb_[:, _ecol(jt):_ecol(jt) + w],
                start=(jt == 0), stop=(jt == JT - 1))
            run("tensor", fn_a)
            if jt == JT - 1:
                p_odone[pp] = inc("tensor", "p", fn_b)
            else:
                run("tensor", fn_b)

        # VECTOR: evacuate o rows + rowsum rows for both pairs
        wait("vector", "p", p_odone[pp])
        if pp >= 1:
            wait("vector", "dr", 64 * pp)  # rs_row slots reuse (all issued)
        for si, pq in enumerate((pa, pb_)):
            run("vector", lambda e, si=si, pq=pq: e.tensor_copy(
                sb["oT"][0:64, pq * 512:(pq + 1) * 512]
                .rearrange("a (h b c) -> a h b c", h=2, b=2),
                opp2[0:64, si * 512:(si + 1) * 512]
                .rearrange("a (b h c) -> a h b c", b=2, h=2)))
        for si, pq in enumerate((pa, pb_)):
            fn = lambda e, si=si: e.tensor_copy(
                sb["rs_row"][64:65, si * 512:(si + 1) * 512]
                .rearrange("a (h b c) -> a h b c", h=2, b=2),
                opp2[64:65, si * 512:(si + 1) * 512]
                .rearrange("a (b h c) -> a h b c", b=2, h=2))
            if si == 1:
                v_oevac[pp] = inc("vector", "v", fn)
            else:
                run("vector", fn)
        # SYNC: scatter rowsums into rs64. Layout: partition q = h*32 + p*4
        # + bc//64, col = bc%64 -- partition-major iteration of rs64 is then
        # exactly the even/odd-head-permuted recipflat order.
        wait("gpsimd", "v", v_oevac[pp])
        for si, pq in enumerate((pa, pb_)):
            for h in range(2):
                dma("dr", sb["rs64"][h * 32 + pq * 4:h * 32 + pq * 4 + 4, :],
                    sb["rs_row"][64:65,
                                 si * 512 + h * 256:si * 512 + (h + 1) * 256],
                    eng="gpsimd")

        # SYNC: ebias prefetch for pairs pa+2, pb_+2 (slot free once both
        # engines' E-mults for the current occupant finish)
        for pq in (pa, pb_):
            if pq + 2 < NPAIR:
                for s in ("v", "g"):
                    ent = [e_mult[pq][c] for c in range(NCH)
                           if e_mult[pq][c][0] == s]
                    if ent:
                        wait("sync", s, ent[-1][1])
                wait("sync", "p", p_simc[pq][4])  # PE reads bh (ck4 bias mms)
                dma(f"db{pq % 2}", bsb[pq % 2][:], d_in["ebias"][pq + 2])

    # ========== tail: reciprocal + normalize + output projection ==========
    wait("vector", "dr", cnt["dr"])
    runD("vector", lambda e: e.reciprocal(out=sb["r64f"][:], in_=sb["rs64"][:]))
    v_recip = inc("vector", "v", lambda e: e.tensor_copy(
        sb["r64b"][:], sb["r64f"][:]))
    # direct SBUF->SBUF flatten: partition-major r64b == permuted order
    wait("gpsimd", "v", v_recip)
    d_m = dma("dm", sb["recipflat"][0:1, :], sb["r64b"][:], eng="gpsimd")

    # PE: broadcast matmuls R = ones64 x recipflat (one LDWEIGHTS total)
    wait("tensor", "dm", d_m)
    wait("tensor", "s", s_exp[NPAIR - 1][NCH - 1])  # simA/simB free
    wait("tensor", "v", v_oevac[NPAIR // 2 - 1])    # opp2 free
    p_bc = [0] * 3
    # R_even (heads 0,2,..14) -> simB (3 x 512) + opp2[0:64, 0:512]
    # R_odd  (heads 1,3,..15) -> simA (3 x 512) + opp2[0:64, 512:1024]
    bc_dsts = ([(simB, i * 512, i * 512) for i in range(3)]
               + [(opp2, 0, 1536)]
               + [(simA, i * 512, 2048 + i * 512) for i in range(3)]
               + [(opp2, 512, 3584)])
    for i, (ps, poff, roff) in enumerate(bc_dsts):
        fn = lambda e, i=i, ps=ps, poff=poff, roff=roff: (
            e.matmul if i == 0 else (lambda *a, **k: mm_reuse(e, *a, **k)))(
            ps[0:64, poff:poff + 512], sb["onesbf"][0:1, 0:64],
            sb["recipflat"][0:1, roff:roff + 512], start=True, stop=True)
        if i in (3, 6, 7):
            p_bc[{3: 0, 6: 1, 7: 2}[i]] = inc("tensor", "p", fn)
        else:
            run("tensor", fn)

    # VECTOR: oTn = oT * R  (even heads -> oTn[0:64], odd -> oTn_lo)
    oT_hp = sb["oT"][0:64, :].rearrange("a (p k c) -> a p k c", p=8, k=2)
    wait("vector", "p", p_bc[0])
    v_n0 = inc("vector", "v", lambda e: e.tensor_tensor(
        out=sb["oTn"][0:64, 0:1536].rearrange("a (p c) -> a p c", p=6),
        in0=oT_hp[:, 0:6, 0, :],
        in1=simB[0:64, 0:1536].rearrange("a (p c) -> a p c", p=6),
        op=OP.mult))
    run("vector", lambda e: e.tensor_tensor(
        out=sb["oTn"][0:64, 1536:2048].rearrange("a (p c) -> a p c", p=2),
        in0=oT_hp[:, 6:8, 0, :],
        in1=opp2[0:64, 0:512].rearrange("a (p c) -> a p c", p=2),
        op=OP.mult))
    wait("vector", "p", p_bc[1])
    run("vector", lambda e: e.tensor_tensor(
        out=sb["oTn_lo"][0:64, 0:1536].rearrange("a (p c) -> a p c", p=6),
        in0=oT_hp[:, 0:6, 1, :],
        in1=simA[0:64, 0:1536].rearrange("a (p c) -> a p c", p=6),
        op=OP.mult))
    wait("vector", "p", p_bc[2])
    v_n2 = inc("vector", "v", lambda e: e.tensor_tensor(
        out=sb["oTn_lo"][0:64, 1536:2048].rearrange("a (p c) -> a p c", p=2),
        in0=oT_hp[:, 6:8, 1, :],
        in1=opp2[0:64, 512:1024].rearrange("a (p c) -> a p c", p=2),
        op=OP.mult))
    wait("gpsimd", "v", v_n2)
    d_n = dma("dn", sb["oTn"][64:128, :], sb["oTn_lo"][0:64, :], eng="gpsimd")

    # ========== output projection (oTn stationary: one load serves 1024
    # wout columns; out lands [query, dim] and the DMA transposes on the
    # DRAM side) ====
    wait("tensor", "v", v_n0)
    wait("tensor", "v", v_n2)   # R reads of simA/simB/opp2 complete
    wait("tensor", "dn", d_n)
    wait("tensor", "dw", d_w)
    # accumulators: (qb, dh) -> 512-f32 bank each
    oreg = [[simA[:, 0:512], simA[:, 512:1024]],
            [simA[:, 1024:1536], simB[:, 0:512]]]
    p_out = [[0, 0], [0, 0]]
    for qb in range(2):
        for hdt in range(8):
            for dh in range(2):
                def fn(e, qb=qb, hdt=hdt, dh=dh):
                    f = e.matmul if dh == 0 else (
                        lambda *a, **k: mm_reuse(e, *a, **k))
                    return f(
                        oreg[qb][dh],
                        sb["oTn"][:, hdt * NQ + qb * 128:hdt * NQ + (qb + 1) * 128],
                        sb["wout"][:, hdt * DIM + dh * 512:hdt * DIM + (dh + 1) * 512],
                        start=(hdt == 0), stop=(hdt == 7))
                if hdt == 7:
                    p_out[qb][dh] = inc("tensor", "p", fn)
                else:
                    run("tensor", fn)
    s_outevac = [[0, 0], [0, 0]]
    for qb in range(2):
        for dh in range(2):
            wait("scalar", "p", p_out[qb][dh])
            oslot = dh
            if qb == 1:
                wait("scalar", "do", 32 * qb)  # outsb slot reuse
            s_outevac[qb][dh] = inc(
                "scalar", "s", lambda e, qb=qb, dh=dh, oslot=oslot:
                e.activation(out=sb["outsb"][:, oslot * 512:(oslot + 1) * 512],
                             in_=oreg[qb][dh], func=AF.Copy))
            wait("sync", "s", s_outevac[qb][dh])
            dma("do",
                out_d[dh * 512:(dh + 1) * 512, qb * 128:(qb + 1) * 128]
                .rearrange("d q -> q d"),
                sb["outsb"][:, oslot * 512:(oslot + 1) * 512])

    # ========== emit ==========
    from contextlib import ExitStack as _ES
    semctx = _ES()
    for k in ("p", "v", "s", "g") + DSEMS:
        SEM[k] = semctx.enter_context(nc.semaphore(f"sem_{k}"))
    with semctx:
        with nc.Block() as block:
            @block.sync
            def _(e):
                for fn in plan["sync"]:
                    fn(e)

            @block.tensor
            def _(e):
                for fn in plan["tensor"]:
                    fn(e)

            @block.vector
            def _(e):
                for fn in plan["vector"]:
                    fn(e)

            @block.scalar
            def _(e):
                for fn in plan["scalar"]:
                    fn(e)

            @block.gpsimd
            def _(e):
                for fn in plan["gpsimd"]:
                    fn(e)
    ctx.close()
    return nc


def _prep_inputs(x, attn_bias, Wq, Wkv, null_kv, Wout, gamma, mask):
    from ml_dtypes import bfloat16
    x = np.asarray(x, np.float32)[0]            # [N, DIM]
    attn_bias = np.asarray(attn_bias, np.float32)[0]  # [H, N, N]
    Wq = np.asarray(Wq, np.float32)
    Wkv = np.asarray(Wkv, np.float32)
    null_kv = np.asarray(null_kv, np.float32)
    Wout = np.asarray(Wout, np.float32)
    gamma = np.asarray(gamma, np.float32)
    mask = np.asarray(mask, bool)[0]            # [N]

    scale = DIM_HEAD ** -0.5
    wq_eff = (gamma[:, None] * Wq * scale).astype(np.float32)
    srow = wq_eff.sum(axis=0, keepdims=True)
    xt = np.ascontiguousarray(x.T)
    nkvt = np.zeros((128, NUM_NULL), np.float32)
    nkvt[0:DIM_HEAD, :] = null_kv[0].T
    nkvt[64:64 + DIM_HEAD, :] = null_kv[1].T
    I128 = np.eye(128, dtype=np.float32)
    ones = np.ones((1, 128), np.float32)

    jpad = np.arange(JPAD)
    jvalid = np.zeros(JPAD, bool)
    jvalid[:NUM_NULL] = True
    jvalid[NUM_NULL:NUM_NULL + N] = mask
    key_of_j = jpad - NUM_NULL

    in_maps = []
    idx_all = []
    for c in range(NCORES):
        idx = np.concatenate([np.arange(c, 1024, 8), np.arange(1024 + c, 2048, 8)])
        idx_all.append(idx)
        allow = jvalid[None, :] & (key_of_j[None, :] <= idx[:, None])  # [NQ, JPAD]
        allow[:, :NUM_NULL] = True
        ab = np.zeros((HEADS, JPAD, NQ), np.float32)
        ab[:, NUM_NULL:NUM_NULL + N, :] = attn_bias[:, idx, :].transpose(0, 2, 1)
        btraw = np.where(allow.T[None], ab, MASK_VAL)  # raw bias+mask
        bt = np.exp(btraw)                             # exp(bias), masked->0
        pk = np.empty((HEADS // 2, 128, EWP), np.float32)
        for jt in range(SH_JT):
            c0 = jt * 512
            tile = bt[:, jt * 128:(jt + 1) * 128, :]        # [H, 128, 256]
            pk[:, :, c0:c0 + 128] = tile[0::2, :, 0:128]         # h0 b0
            pk[:, :, c0 + 128:c0 + 256] = tile[1::2, :, 0:128]   # h1 b0
            pk[:, :, c0 + 256:c0 + 384] = tile[0::2, :, 128:256]  # h0 b1
            pk[:, :, c0 + 384:c0 + 512] = tile[1::2, :, 128:256]  # h1 b1
        for jt in range(SH_JT, JT):
            c0 = SH_JT * 512 + (jt - SH_JT) * 256
            # jt >= 13 (psum chunk 4): RAW additive bias, added on PE
            src = bt if jt < 13 else btraw
            tile = src[:, jt * 128:(jt + 1) * 128, 128:256]  # [H, 128, 128]
            pk[:, :, c0:c0 + 128] = tile[0::2]
            pk[:, :, c0 + 128:c0 + 256] = tile[1::2]
        in_maps.append({
            "xtq": np.ascontiguousarray(xt[:, idx]).astype(bfloat16),
            "xt": xt.astype(bfloat16),
            "wq": wq_eff.astype(bfloat16),
            "srow": srow.astype(bfloat16),
            "wkv": Wkv.astype(bfloat16),
            "nkvt": nkvt.astype(bfloat16),
            "wout": Wout.astype(bfloat16),
            "ibf": I128.astype(bfloat16),
            "onesbf": ones.astype(bfloat16),
            "onesf": ones,
            "ebias": pk.astype(bfloat16),
        })
    return in_maps, idx_all


def _run(inputs, trace=False):
    from concourse.bass_utils import run_bass_kernel_spmd
    if "nc" not in _CACHE:
        _CACHE["nc"] = _build_graph()
    nc = _CACHE["nc"]
    in_maps, idx_all = _prep_inputs(**inputs)
    res = run_bass_kernel_spmd(nc, in_maps, list(range(NCORES)), trace=trace)
    out = np.zeros((B, N, DIM), np.float32)
    for c in range(NCORES):
        out[0, idx_all[c], :] = res.results[c]["out"].T
    return out, res


def kernel(**inputs):
    out, _ = _run(inputs, trace=False)
    return out


# revision 51
# speedup vs baseline: 1.1361x; 1.1361x over previous
"""Trainium2 8-core kernel for nn_Attention_70892730187933 (sparse multi-query attention).

Sharding: sequence-parallel over query rows. Core c owns rows {i : i % 8 == c},
as 2 blocks of 128 rows (block0 < 1024, block1 >= 1024). Causal trimming:
block0 needs key j-tiles 0..8, block1 needs 0..16 (key space padded to
17*128 = 2176 incl. 2 null cols). No collectives; host concatenates rows.

Activations stay transposed [feature, token] so every matmul contraction dim
lands on partitions with no on-device activation transposes. LayerNorm stats
come from PE ones-column matmuls on bf16 x^T (raw moments); the mean
correction folds into the Q projection (rank-1 srow term). The bias+mask is
applied MULTIPLICATIVELY: host precomputes exp(bias+mask) (masked -> 0), and
after ScalarE computes exp(sim) from PSUM, DVE multiplies it in bf16 -- no
per-tile identity bias matmuls on the Tensor
engine. PE stationary loads (LDWEIGHTS ~145ns each) are halved where tiles
repeat: stats/broadcast ones vectors, kv projection run ct-outer (one wkv
load serves 4 column chunks), and o-matmuls process TWO head-pairs jointly
(one V-tile load serves both pairs' E) via ldweights=False on the repeats.
Rowsums ride as a ones-column in V; per-pair SBUF->SBUF DMAs scatter them
into a [64, 64] tile laid out so a direct flatten DMA yields the even/odd-
head-permuted reciprocal row for 8 PE broadcast matmuls + 3 normalize mults.

Raw Block + explicit semaphores: this walrus build rejects instructions with
multiple attached sync waits, so Tile-generated sync cannot compile; every
cross-engine wait here is its own sequencer instruction. The builder plans all
five engine programs in one pass (semaphore counters known at plan time), then
emits them inside one Block.
"""

import sys
import numpy as np

sys.path.insert(0, "/opt/trn_rl_repo")

B, N, DIM, HEADS, DIM_HEAD, NUM_NULL = 1, 2048, 1024, 16, 64, 2
INNER = HEADS * DIM_HEAD
EPS = 1e-5
NCORES = 8
JT = 17
JPAD = JT * 128
NQ = 256
MASK_VAL = -30000.0
SH_JT = 9
NPAIR = HEADS // 2
EWP = SH_JT * 512 + (JT - SH_JT) * 256  # 6656: packed pair E/ebias width
# psum chunks: 5 per pair so two [65,512] o-accumulators fit alongside.
# A-region: ck0,ck2,ck4 (max 1536); B-region: ck1,ck3 (max 1536).
CH_OF_JT = [0, 0, 0, 1, 1, 1, 2, 2, 2, 3, 3, 3, 3, 4, 4, 4, 4]
CH_LEN = [1536, 1536, 1536, 1024, 1024]
CH_ECOL = [0, 1536, 3072, 4608, 5632]
CH_ENG = ["vector", "vector", "vector", "vector", "vector"]  # E-mult engine
NCH = 5

_CACHE = {}


def _ecol(jt):
    """Column of j-tile jt in packed pair E/ebias layout."""
    return jt * 512 if jt < SH_JT else SH_JT * 512 + (jt - SH_JT) * 256


def _ccol(jt):
    """Column of j-tile jt inside its psum chunk."""
    if jt < SH_JT:
        return (jt % 3) * 512
    if jt < 13:
        return (jt - 9) * 256
    return (jt - 13) * 256


def _build_graph():
    from contextlib import ExitStack
    import concourse.bass as bass
    import concourse.mybir as mybir

    dt = mybir.dt
    F32, BF16 = dt.float32, dt.bfloat16
    AF = mybir.ActivationFunctionType
    OP = mybir.AluOpType
    nc = bass.Bass()

    d_in = {}
    for name, shape, ty in [
        ("xtq", [DIM, NQ], BF16), ("xt", [DIM, N], BF16),
        ("wq", [DIM, INNER], BF16), ("srow", [1, INNER], BF16),
        ("wkv", [DIM, 2 * DIM_HEAD], BF16), ("nkvt", [128, NUM_NULL], BF16),
        ("wout", [INNER, DIM], BF16), ("ibf", [128, 128], BF16),
        ("onesbf", [1, 128], BF16), ("onesf", [1, 128], F32),
        ("ebias", [NPAIR, 128, EWP], BF16),
    ]:
        d_in[name] = nc.declare_dram_parameter(name, shape, ty, isOutput=False)
    out_d = nc.declare_dram_parameter("out", [DIM, NQ], F32, isOutput=True)

    ctx = ExitStack()
    sb = {}
    for name, shape, ty in [
        ("ibf", [128, 128], BF16), ("onesbf", [1, 128], BF16),
        ("onesf", [1, 128], F32), ("srow", [1, INNER], BF16),
        ("zb", [128, 1], F32), ("epsb", [128, 1], F32),
        ("onescol", [128, 1], BF16),
        ("wq", [128, 8 * INNER], BF16), ("wout", [128, 8 * DIM], BF16),
        ("wkv", [128, 8 * 128], BF16), ("xt", [128, 8 * N], BF16),
        ("xtq", [128, 8 * NQ], BF16), ("xsq", [128, 8 * NQ], BF16),
        ("lnrow", [1, 1024], F32),  # 0:256 negmu, 256:512 ex2, 512:768 var, 768:1024 rsq
        ("negmurs", [1, NQ], BF16),
        ("xst", [128, 8 * NQ], BF16), ("qtmp", [128, 2 * NQ], BF16),
        ("kv", [128, JPAD], BF16), ("vsb", [128, JT * 65], BF16),
        ("e0", [128, EWP], BF16), ("e1", [128, EWP], BF16),
        ("b0", [128, EWP], BF16), ("b1", [128, EWP], BF16),
        ("oT", [64, HEADS * NQ], BF16), ("rs_row", [65, 2 * 512], F32),
        ("rs64", [64, 64], F32), ("r64f", [64, 64], F32), ("r64b", [64, 64], BF16),
        ("recipflat", [1, HEADS * NQ], BF16),
        ("oTn", [128, 8 * NQ], BF16), ("oTn_lo", [64, 8 * NQ], BF16),
        ("outsb", [128, 2 * NQ], F32),
    ] + [(f"qh{h}", [64, 2 * NQ], BF16) for h in range(NPAIR)]:
        sb[name] = ctx.enter_context(nc.sbuf_tensor("sb_" + name, shape, ty))

    qh = [sb[f"qh{h}"] for h in range(NPAIR)]
    esb = [sb["e0"], sb["e1"]]
    bsb = [sb["b0"], sb["b1"]]

    # PSUM: early tensors freed before head-loop tensors are allocated.
    early = ExitStack()
    kvp = early.enter_context(nc.psum_tensor("kvp", [128, 1024], F32))
    qp = [early.enter_context(nc.psum_tensor(f"qp{i}", [128, NQ], F32))
          for i in range(2)]
    vp = [early.enter_context(nc.psum_tensor(f"vp{i}", [128, 64], BF16))
          for i in range(2)]
    stp2 = early.enter_context(nc.psum_tensor("stp2", [1, 512], F32))
    rbp = early.enter_context(nc.psum_tensor("rbp", [128, NQ], F32))
    early.close()
    simA = ctx.enter_context(nc.psum_tensor("simA", [128, 1536], F32))
    simB = ctx.enter_context(nc.psum_tensor("simB", [128, 1536], F32))
    opp2 = ctx.enter_context(nc.psum_tensor("opp2", [65, 1024], F32))

    # ------- planner -------
    plan = {"sync": [], "tensor": [], "vector": [], "scalar": [], "gpsimd": []}
    DSEMS = ("dk", "dxq", "dqx", "dsm", "dw",
             "db0", "db1", "dq", "dr", "dm", "dn", "do")
    cnt = {"p": 0, "v": 0, "s": 0, "g": 0, **{k: 0 for k in DSEMS}}
    SEM = {}
    ESEM = {"vector": "v", "gpsimd": "g", "scalar": "s"}

    def wait(eng, sem, thr):
        if thr > 0:
            plan[eng].append(lambda e, s=sem, t=thr: e.wait_ge(SEM[s], t))

    def dma(sem, out, in_, eng="sync"):
        cnt[sem] += 16
        plan[eng].append(
            lambda e, s=sem, o=out, i=in_: e.dma_start(out=o, in_=i)
            .then_inc(SEM[s], 16))
        return cnt[sem]

    def inc(eng, sem, fn):
        # DVE/ACT/Pool pipelines expose writes only after a drain; put the sem
        # update on the drain so consumers see committed data.
        cnt[sem] += 1
        if eng in ("vector", "scalar", "gpsimd"):
            plan[eng].append(lambda e, f=fn: f(e))
            plan[eng].append(lambda e, s=sem: e.drain().then_inc(SEM[s], 1))
        else:
            plan[eng].append(lambda e, f=fn, s=sem: f(e).then_inc(SEM[s], 1))
        return cnt[sem]

    def run(eng, fn):
        plan[eng].append(fn)

    def runD(eng, fn):
        # run + drain, for same-engine RAW chains without a semaphore inc
        plan[eng].append(fn)
        plan[eng].append(lambda e: e.drain())

    def mm_reuse(e, *args, **kwargs):
        # matmul whose stationary was loaded by the immediately preceding
        # matmul on PE: suppress this instruction's LDWEIGHTS.
        m = e.matmul(*args, **kwargs)
        m.ins.ldweights = False
        return m

    # ========== DMA issue order (sync queue = priority order) ==========
    def dma8(sem, nm, eng="sync"):
        return dma(sem, sb[nm][:].rearrange("p (c f) -> p c f", c=8),
                   d_in[nm][:].rearrange("(c p) f -> p c f", c=8), eng=eng)

    d_xq = dma8("dxq", "xtq")
    dma8("dqx", "wq")
    d_qx = cnt["dqx"]
    for nm in ("ibf", "onesbf", "onesf", "srow"):
        dma("dsm", sb[nm][:], d_in[nm][:])
    d_sm0 = dma("dsm", sb["kv"][:, 0:NUM_NULL], d_in["nkvt"][:])
    dma8("dk", "wkv")
    dma8("dk", "xt")
    d_kv = cnt["dk"]
    # NOTE: eb0/eb1/wout are issued AFTER the qproj odd-half DMAs (below) so
    # those small latency-critical transfers aren't queued behind 5.5MB.

    # ========== VECTOR: memsets ==========
    run("vector", lambda e: e.memset(sb["zb"][:], 0.0))
    run("vector", lambda e: e.memset(sb["epsb"][:], EPS))
    run("vector", lambda e: e.memset(sb["onescol"][:], 1.0))
    run("vector", lambda e: e.memset(sb["vsb"][:], 1.0))
    v_memset = inc("vector", "v",
                   lambda e: e.memset(sb["kv"][:, NUM_NULL + N:JPAD], 0.0))

    # ========== LN stats (row layout, raw moments) ==========
    wait("scalar", "dxq", d_xq)
    wait("scalar", "v", v_memset)  # zb ready
    s_xsq = inc("scalar", "s", lambda e: e.activation(
        out=sb["xsq"][:], in_=sb["xtq"][:], func=AF.Square, bias=sb["zb"][:]))

    # tensor: musum (cols 0:256) and sqsum (cols 256:512) into stp2;
    # all 16 matmuls share the onescol stationary (one LDWEIGHTS).
    wait("tensor", "dxq", d_xq)
    wait("tensor", "v", v_memset)  # onescol ready
    for ct in range(8):
        def fn(e, ct=ct):
            f = e.matmul if ct == 0 else (lambda *a, **k: mm_reuse(e, *a, **k))
            return f(stp2[0:1, 0:NQ], sb["onescol"][:],
                     sb["xtq"][:, ct * NQ:(ct + 1) * NQ],
                     start=(ct == 0), stop=(ct == 7))
        if ct == 7:
            p_mu = inc("tensor", "p", fn)
        else:
            run("tensor", fn)

    wait("tensor", "s", s_xsq)
    for ct in range(8):
        fn = lambda e, ct=ct: mm_reuse(
            e, stp2[0:1, NQ:2 * NQ], sb["onescol"][:],
            sb["xsq"][:, ct * NQ:(ct + 1) * NQ],
            start=(ct == 0), stop=(ct == 7))
        if ct == 7:
            p_sq = inc("tensor", "p", fn)
        else:
            run("tensor", fn)

    # vector rowops (RAW chains -> drained)
    neg_mu = sb["lnrow"][0:1, 0:256]
    ex2 = sb["lnrow"][0:1, 256:512]
    var_r = sb["lnrow"][0:1, 512:768]
    rsq_r = sb["lnrow"][0:1, 768:1024]
    wait("vector", "p", p_mu)
    runD("vector", lambda e: e.tensor_scalar_mul(
        out=neg_mu, in0=stp2[0:1, 0:NQ], scalar1=-1.0 / DIM))
    wait("vector", "p", p_sq)
    runD("vector", lambda e: e.tensor_scalar_mul(
        out=ex2, in0=stp2[0:1, NQ:2 * NQ], scalar1=1.0 / DIM))
    runD("vector", lambda e: e.tensor_tensor(
        out=var_r, in0=neg_mu, in1=neg_mu, op=OP.mult))
    v_var = inc("vector", "v", lambda e: e.tensor_tensor(
        out=var_r, in0=ex2, in1=var_r, op=OP.subtract))
    # scalar: rsq = exp(-0.5 * ln(var + eps))
    wait("scalar", "v", v_var)
    runD("scalar", lambda e: e.activation(
        out=var_r, in_=var_r, func=AF.Ln, scale=1.0, bias=sb["epsb"][0:1, :]))
    s_rsq = inc("scalar", "s", lambda e: e.activation(
        out=rsq_r, in_=var_r, func=AF.Exp, scale=-0.5, bias=sb["zb"][0:1, :]))
    wait("vector", "s", s_rsq)
    v_nmr = inc("vector", "v", lambda e: e.tensor_tensor(
        out=sb["negmurs"][:], in0=neg_mu, in1=rsq_r, op=OP.mult))

    # tensor: rsqb broadcast [128, 256] (f32 matmul)
    wait("tensor", "dsm", d_sm0)  # onesf (+ibf/srow/nkvt) loaded
    wait("tensor", "s", s_rsq)
    p_rsqb = inc("tensor", "p", lambda e: e.matmul(
        rbp[:], sb["onesf"][0:1, :], rsq_r, start=True, stop=True))

    # vector: xst = xtq * rsqb
    wait("vector", "p", p_rsqb)
    for ct in range(8):
        fn = lambda e, ct=ct: e.tensor_tensor(
            out=sb["xst"][:, ct * NQ:(ct + 1) * NQ],
            in0=sb["xtq"][:, ct * NQ:(ct + 1) * NQ], in1=rbp[:], op=OP.mult)
        if ct == 7:
            v_xst = inc("vector", "v", fn)
        else:
            run("vector", fn)

    # ========== TENSOR: q projection (qp double-buffered) ==========
    wait("tensor", "v", v_xst)
    wait("tensor", "dqx", d_qx)
    wait("tensor", "v", v_nmr)
    p_q = [0] * 8
    v_qtmp = [0] * 8
    for dtile in range(8):
        pb = qp[dtile % 2]
        if dtile >= 2:
            wait("tensor", "v", v_qtmp[dtile - 2])
        for ct in range(8):
            run("tensor", lambda e, pb=pb, dtile=dtile, ct=ct: e.matmul(
                pb[:],
                sb["wq"][:, ct * INNER + dtile * 128:ct * INNER + (dtile + 1) * 128],
                sb["xst"][:, ct * NQ:(ct + 1) * NQ],
                start=(ct == 0), stop=False))
        p_q[dtile] = inc("tensor", "p", lambda e, pb=pb, dtile=dtile: e.matmul(
            pb[:], sb["srow"][0:1, dtile * 128:(dtile + 1) * 128],
            sb["negmurs"][:], start=False, stop=True))
        wait("vector", "p", p_q[dtile])
        run("vector", lambda e, pb=pb, dtile=dtile: e.tensor_copy(
            qh[dtile][0:64, :].rearrange("a (b h c) -> a b h c", b=2, h=2)[:, :, 0, :],
            pb[0:64, :].rearrange("a (b c) -> a b c", b=2)))
        slot = dtile % 2
        if dtile >= 2:
            wait("vector", "dq", 16 * dtile)  # qtmp slot reuse (all issued)
        v_qtmp[dtile] = inc("vector", "v", lambda e, pb=pb, slot=slot:
                            e.tensor_copy(
                                sb["qtmp"][64:128, slot * NQ:(slot + 1) * NQ],
                                pb[64:128, :]))
        wait("sync", "v", v_qtmp[dtile])
        dma("dq",
            qh[dtile][0:64, :].rearrange("a (b h c) -> a b h c", b=2, h=2)[:, :, 1, :],
            sb["qtmp"][64:128, slot * NQ:(slot + 1) * NQ]
            .rearrange("a (b c) -> a b c", b=2))

    # big loads not needed until the pair loop / tail: issued after the dq
    # DMAs so they sit BEHIND them in the sync DMA queue
    for p in range(2):
        dma(f"db{p}", bsb[p][:], d_in["ebias"][p])
    d_w = dma8("dw", "wout")

    # ========== TENSOR: kv matmuls, ct-outer (one wkv load per ct) ==========
    wait("tensor", "dk", d_kv)
    s_kvevac = [0] * 4
    p_kvh = [0, 0]
    for half in range(2):
        if half == 1:
            wait("tensor", "s", s_kvevac[1])  # kvp reuse
        for ct in range(8):
            for chh in range(2):
                ch = half * 2 + chh
                def fn(e, ct=ct, ch=ch, chh=chh):
                    f = e.matmul if chh == 0 else (
                        lambda *a, **k: mm_reuse(e, *a, **k))
                    return f(
                        kvp[:, chh * 512:(chh + 1) * 512],
                        sb["wkv"][:, ct * 128:(ct + 1) * 128],
                        sb["xt"][:, ct * N + ch * 512:ct * N + (ch + 1) * 512],
                        start=(ct == 0), stop=(ct == 7))
                if ct == 7 and chh == 1:
                    p_kvh[half] = inc("tensor", "p", fn)
                else:
                    run("tensor", fn)
        wait("scalar", "p", p_kvh[half])
        for chh in range(2):
            ch = half * 2 + chh
            s_kvevac[ch] = inc("scalar", "s", lambda e, ch=ch, chh=chh:
                               e.activation(
                                   out=sb["kv"][:, NUM_NULL + ch * 512:
                                                NUM_NULL + (ch + 1) * 512],
                                   in_=kvp[:, chh * 512:(chh + 1) * 512],
                                   func=AF.Copy))

    # ========== TENSOR: v transposes (vp double-buffered) ==========
    p_vt = [0] * JT
    v_vcopy = [0] * JT
    for jt in range(JT):
        pb = vp[jt % 2]
        ch_hi = min(3, ((jt + 1) * 128 - 1 - NUM_NULL) // 512)
        wait("tensor", "s", s_kvevac[ch_hi])
        if jt == JT - 1:
            wait("tensor", "v", v_memset)
        if jt >= 2:
            wait("tensor", "v", v_vcopy[jt - 2])
        p_vt[jt] = inc("tensor", "p", lambda e, pb=pb, jt=jt: e.transpose(
            pb[:], sb["kv"][64:128, jt * 128:(jt + 1) * 128],
            sb["ibf"][64:128, 64:128]))
        wait("vector", "p", p_vt[jt])
        v_vcopy[jt] = inc("vector", "v", lambda e, pb=pb, jt=jt: e.tensor_copy(
            sb["vsb"][:, jt * 65:jt * 65 + 64], pb[:]))
    v_vsb = v_vcopy[JT - 1]

    # ========== PAIR LOOP: sims per pair, o-matmuls joint per pair-pair ====
    v_pre_heads = cnt["v"]
    s_exp = [[0] * NCH for _ in range(NPAIR)]
    e_mult = [[None] * NCH for _ in range(NPAIR)]  # (sem_name, count)
    p_simc = [[0] * NCH for _ in range(NPAIR)]
    p_odone = [0] * (NPAIR // 2)
    v_oevac = [0] * (NPAIR // 2)

    wait("tensor", "s", s_kvevac[3])
    wait("tensor", "v", v_pre_heads)  # early-psum reuse guard
    wait("tensor", "dq", 16 * 8)      # all odd-half q DMAs done

    def emit_pair_sims(p):
        """sims + exp + E-mult for pair p (5 chunks)."""
        eh = esb[p % 2]
        bh = bsb[p % 2]
        qpair = qh[p][0:64, :]
        qsolo = qh[p][0:64, NQ:2 * NQ]
        for ck in range(NCH):
            ps = simA if ck in (0, 2, 4) else simB
            jts = [jt for jt in range(JT) if CH_OF_JT[jt] == ck]
            # psum region reuse: wait on exp of the previous occupant
            if ck in (0, 1):
                if p >= 1:
                    wait("tensor", "s", s_exp[p - 1][{0: 4, 1: 3}[ck]])
            else:
                wait("tensor", "s", s_exp[p][ck - 2])
            if ck != 4:
                for jt in jts:
                    w = 512 if jt < SH_JT else 256
                    rhs = qpair if jt < SH_JT else qsolo
                    fn = lambda e, ps=ps, jt=jt, r=rhs, w=w: e.matmul(
                        ps[:, _ccol(jt):_ccol(jt) + w],
                        sb["kv"][0:64, jt * 128:(jt + 1) * 128], r,
                        start=True, stop=True)
                    if jt == jts[-1]:
                        p_simc[p][ck] = inc("tensor", "p", fn)
                    else:
                        run("tensor", fn)
            else:
                # last chunk: additive bias via identity matmuls on PE (host
                # packs RAW bias+mask here, not exp) -- exp then yields final
                # E with no DVE hop on the joint-o critical path. Group j-
                # tiles bank-pairwise: only one open accumulation group per
                # psum bank is allowed.
                wait("tensor", f"db{p % 2}", 16 * (p // 2 + 1))
                for grp in ((13, 15), (14, 16)):
                    for jt in grp:
                        run("tensor", lambda e, ps=ps, jt=jt: e.matmul(
                            ps[:, _ccol(jt):_ccol(jt) + 256],
                            sb["kv"][0:64, jt * 128:(jt + 1) * 128], qsolo,
                            start=True, stop=False))
                    for ji, jt in enumerate(grp):
                        def fnb(e, jt=jt, ji=ji, ps=ps):
                            f = e.matmul if ji == 0 else (
                                lambda *a, **k: mm_reuse(e, *a, **k))
                            return f(ps[:, _ccol(jt):_ccol(jt) + 256],
                                     sb["ibf"][:],
                                     bh[:, _ecol(jt):_ecol(jt) + 256],
                                     start=False, stop=True)
                        if grp == (14, 16) and ji == 1:
                            p_simc[p][ck] = inc("tensor", "p", fnb)
                        else:
                            run("tensor", fnb)
            # SCALAR: exp for this chunk
            wait("scalar", "p", p_simc[p][ck])
            if ck == 0 and p >= 2:
                wait("scalar", "p", p_odone[p // 2 - 1])  # E slot reuse
            ln = CH_LEN[ck]
            s_exp[p][ck] = inc("scalar", "s", lambda e, ps=ps, ck=ck, ln=ln,
                               eh=eh: e.activation(
                                   out=eh[:, CH_ECOL[ck]:CH_ECOL[ck] + ln],
                                   in_=ps[:, 0:ln], func=AF.Exp, bias=sb["zb"][:]))
            if ck == 4:
                # bias already added in PSUM; exp output IS the final E
                e_mult[p][ck] = ("s", s_exp[p][ck])
                continue
            # DVE: E *= exp(bias) in place (bf16, all-SBUF)
            eng = CH_ENG[ck]
            sem = ESEM[eng]
            if ck in (0, 1):  # first chunk on each engine: ebias slot loaded
                wait(eng, f"db{p % 2}", 16 * (p // 2 + 1))
            wait(eng, "s", s_exp[p][ck])
            n = inc(eng, sem, lambda e, ck=ck, ln=ln, eh=eh, bh=bh:
                    e.tensor_tensor(
                        out=eh[:, CH_ECOL[ck]:CH_ECOL[ck] + ln],
                        in0=eh[:, CH_ECOL[ck]:CH_ECOL[ck] + ln],
                        in1=bh[:, CH_ECOL[ck]:CH_ECOL[ck] + ln],
                        op=OP.mult))
            e_mult[p][ck] = (sem, n)

    for pp in range(NPAIR // 2):
        pa, pb_ = 2 * pp, 2 * pp + 1
        emit_pair_sims(pa)
        emit_pair_sims(pb_)

        # TENSOR: joint o-matmuls for pairs (pa, pb_): per jt one V-tile
        # load serves both pairs (second matmul reuses the stationary).
        if pp == 0:
            wait("tensor", "v", v_vsb)
        if pp >= 1:
            wait("tensor", "v", v_oevac[pp - 1])  # opp2 reuse
        ea, eb_ = esb[0], esb[1]
        for jt in range(JT):
            ck = CH_OF_JT[jt]
            if jt == 0 or _ccol(jt) == 0:
                for pq in (pa, pb_):
                    sem, n = e_mult[pq][ck]
                    wait("tensor", sem, n)
            w, eoff = (512, 0) if jt < SH_JT else (256, 256)
            fn_a = lambda e, jt=jt, w=w, eoff=eoff: e.matmul(
                opp2[0:65, eoff:eoff + w],
                sb["vsb"][:, jt * 65:jt * 65 + 65],
                ea[:, _ecol(jt):_ecol(jt) + w],
                start=(jt == 0), stop=(jt == JT - 1))
            fn_b = lambda e, jt=jt, w=w, eoff=eoff: mm_reuse(
                e, opp2[0:65, 512 + eoff:512 + eoff + w],
                sb["vsb"][:, jt * 65:jt * 65 + 65],
                eb_[:, _ecol(jt):_ecol(jt) + w],
                start=(jt == 0), stop=(jt == JT - 1))
            run("tensor", fn_a)
            if jt == JT - 1:
                p_odone[pp] = inc("tensor", "p", fn_b)
            else:
                run("tensor", fn_b)

        # VECTOR: evacuate o rows + rowsum rows for both pairs
        wait("vector", "p", p_odone[pp])
        if pp >= 1:
            wait("vector", "dr", 64 * pp)  # rs_row slots reuse (all issued)
        for si, pq in enumerate((pa, pb_)):
            run("vector", lambda e, si=si, pq=pq: e.tensor_copy(
                sb["oT"][0:64, pq * 512:(pq + 1) * 512]
                .rearrange("a (h b c) -> a h b c", h=2, b=2),
                opp2[0:64, si * 512:(si + 1) * 512]
                .rearrange("a (b h c) -> a h b c", b=2, h=2)))
        for si, pq in enumerate((pa, pb_)):
            fn = lambda e, si=si: e.tensor_copy(
                sb["rs_row"][64:65, si * 512:(si + 1) * 512]
                .rearrange("a (h b c) -> a h b c", h=2, b=2),
                opp2[64:65, si * 512:(si + 1) * 512]
                .rearrange("a (b h c) -> a h b c", b=2, h=2))
            if si == 1:
                v_oevac[pp] = inc("vector", "v", fn)
            else:
                run("vector", fn)
        # SYNC: scatter rowsums into rs64. Layout: partition q = h*32 + p*4
        # + bc//64, col = bc%64 -- partition-major iteration of rs64 is then
        # exactly the even/odd-head-permuted recipflat order.
        wait("sync", "v", v_oevac[pp])
        for si, pq in enumerate((pa, pb_)):
            for h in range(2):
                dma("dr", sb["rs64"][h * 32 + pq * 4:h * 32 + pq * 4 + 4, :],
                    sb["rs_row"][64:65,
                                 si * 512 + h * 256:si * 512 + (h + 1) * 256])

        # SYNC: ebias prefetch for pairs pa+2, pb_+2 (slot free once both
        # engines' E-mults for the current occupant finish)
        for pq in (pa, pb_):
            if pq + 2 < NPAIR:
                for s in ("v", "g"):
                    ent = [e_mult[pq][c] for c in range(NCH)
                           if e_mult[pq][c][0] == s]
                    if ent:
                        wait("sync", s, ent[-1][1])
                wait("sync", "p", p_simc[pq][4])  # PE reads bh (ck4 bias mms)
                dma(f"db{pq % 2}", bsb[pq % 2][:], d_in["ebias"][pq + 2])

    # ========== tail: reciprocal + normalize + output projection ==========
    wait("vector", "dr", cnt["dr"])
    runD("vector", lambda e: e.reciprocal(out=sb["r64f"][:], in_=sb["rs64"][:]))
    v_recip = inc("vector", "v", lambda e: e.tensor_copy(
        sb["r64b"][:], sb["r64f"][:]))
    # direct SBUF->SBUF flatten: partition-major r64b == permuted order
    wait("sync", "v", v_recip)
    d_m = dma("dm", sb["recipflat"][0:1, :], sb["r64b"][:])

    # PE: broadcast matmuls R = ones64 x recipflat (one LDWEIGHTS total)
    wait("tensor", "dm", d_m)
    wait("tensor", "s", s_exp[NPAIR - 1][NCH - 1])  # simA/simB free
    wait("tensor", "v", v_oevac[NPAIR // 2 - 1])    # opp2 free
    p_bc = [0] * 3
    # R_even (heads 0,2,..14) -> simB (3 x 512) + opp2[0:64, 0:512]
    # R_odd  (heads 1,3,..15) -> simA (3 x 512) + opp2[0:64, 512:1024]
    bc_dsts = ([(simB, i * 512, i * 512) for i in range(3)]
               + [(opp2, 0, 1536)]
               + [(simA, i * 512, 2048 + i * 512) for i in range(3)]
               + [(opp2, 512, 3584)])
    for i, (ps, poff, roff) in enumerate(bc_dsts):
        fn = lambda e, i=i, ps=ps, poff=poff, roff=roff: (
            e.matmul if i == 0 else (lambda *a, **k: mm_reuse(e, *a, **k)))(
            ps[0:64, poff:poff + 512], sb["onesbf"][0:1, 0:64],
            sb["recipflat"][0:1, roff:roff + 512], start=True, stop=True)
        if i in (3, 6, 7):
            p_bc[{3: 0, 6: 1, 7: 2}[i]] = inc("tensor", "p", fn)
        else:
            run("tensor", fn)

    # VECTOR: oTn = oT * R  (even heads -> oTn[0:64], odd -> oTn_lo)
    oT_hp = sb["oT"][0:64, :].rearrange("a (p k c) -> a p k c", p=8, k=2)
    wait("vector", "p", p_bc[0])
    v_n0 = inc("vector", "v", lambda e: e.tensor_tensor(
        out=sb["oTn"][0:64, 0:1536].rearrange("a (p c) -> a p c", p=6),
        in0=oT_hp[:, 0:6, 0, :],
        in1=simB[0:64, 0:1536].rearrange("a (p c) -> a p c", p=6),
        op=OP.mult))
    run("vector", lambda e: e.tensor_tensor(
        out=sb["oTn"][0:64, 1536:2048].rearrange("a (p c) -> a p c", p=2),
        in0=oT_hp[:, 6:8, 0, :],
        in1=opp2[0:64, 0:512].rearrange("a (p c) -> a p c", p=2),
        op=OP.mult))
    wait("vector", "p", p_bc[1])
    run("vector", lambda e: e.tensor_tensor(
        out=sb["oTn_lo"][0:64, 0:1536].rearrange("a (p c) -> a p c", p=6),
        in0=oT_hp[:, 0:6, 1, :],
        in1=simA[0:64, 0:1536].rearrange("a (p c) -> a p c", p=6),
        op=OP.mult))
    wait("vector", "p", p_bc[2])
    v_n2 = inc("vector", "v", lambda e: e.tensor_tensor(
        out=sb["oTn_lo"][0:64, 1536:2048].rearrange("a (p c) -> a p c", p=2),
        in0=oT_hp[:, 6:8, 1, :],
        in1=opp2[0:64, 512:1024].rearrange("a (p c) -> a p c", p=2),
        op=OP.mult))
    wait("sync", "v", v_n2)
    d_n = dma("dn", sb["oTn"][64:128, :], sb["oTn_lo"][0:64, :])

    # ========== output projection (accumulators alternate in simB banks) ====
    wait("tensor", "v", v_n0)
    wait("tensor", "dn", d_n)
    wait("tensor", "dw", d_w)
    s_outevac = [0] * 8
    for et in range(8):
        reg = simB[:, 0:NQ] if et % 2 == 0 else simB[:, 512:512 + NQ]
        if et >= 2:
            wait("tensor", "s", s_outevac[et - 2])
        p_wout = 0
        for hdt in range(8):
            fn = lambda e, et=et, hdt=hdt, reg=reg: e.matmul(
                reg, sb["wout"][:, hdt * DIM + et * 128:hdt * DIM + (et + 1) * 128],
                sb["oTn"][:, hdt * NQ:(hdt + 1) * NQ],
                start=(hdt == 0), stop=(hdt == 7))
            if hdt == 7:
                p_wout = inc("tensor", "p", fn)
            else:
                run("tensor", fn)
        wait("scalar", "p", p_wout)
        oslot = et % 2
        if et >= 2:
            wait("scalar", "do", 16 * et)  # outsb slot reuse (all issued)
        s_outevac[et] = inc("scalar", "s", lambda e, oslot=oslot, reg=reg:
                            e.activation(
                                out=sb["outsb"][:, oslot * NQ:(oslot + 1) * NQ],
                                in_=reg, func=AF.Copy))
        wait("sync", "s", s_outevac[et])
        dma("do", out_d[et * 128:(et + 1) * 128, :],
            sb["outsb"][:, oslot * NQ:(oslot + 1) * NQ])

    # ========== emit ==========
    from contextlib import ExitStack as _ES
    semctx = _ES()
    for k in ("p", "v", "s", "g") + DSEMS:
        SEM[k] = semctx.enter_context(nc.semaphore(f"sem_{k}"))
    with semctx:
        with nc.Block() as block:
            @block.sync
            def _(e):
                for fn in plan["sync"]:
                    fn(e)

            @block.tensor
            def _(e):
                for fn in plan["tensor"]:
                    fn(e)

            @block.vector
            def _(e):
                for fn in plan["vector"]:
                    fn(e)

            @block.scalar
            def _(e):
                for fn in plan["scalar"]:
                    fn(e)

            @block.gpsimd
            def _(e):
                for fn in plan["gpsimd"]:
                    fn(e)
    ctx.close()
    return nc


def _prep_inputs(x, attn_bias, Wq, Wkv, null_kv, Wout, gamma, mask):
    from ml_dtypes import bfloat16
    x = np.asarray(x, np.float32)[0]            # [N, DIM]
    attn_bias = np.asarray(attn_bias, np.float32)[0]  # [H, N, N]
    Wq = np.asarray(Wq, np.float32)
    Wkv = np.asarray(Wkv, np.float32)
    null_kv = np.asarray(null_kv, np.float32)
    Wout = np.asarray(Wout, np.float32)
    gamma = np.asarray(gamma, np.float32)
    mask = np.asarray(mask, bool)[0]            # [N]

    scale = DIM_HEAD ** -0.5
    wq_eff = (gamma[:, None] * Wq * scale).astype(np.float32)
    srow = wq_eff.sum(axis=0, keepdims=True)
    xt = np.ascontiguousarray(x.T)
    nkvt = np.zeros((128, NUM_NULL), np.float32)
    nkvt[0:DIM_HEAD, :] = null_kv[0].T
    nkvt[64:64 + DIM_HEAD, :] = null_kv[1].T
    I128 = np.eye(128, dtype=np.float32)
    ones = np.ones((1, 128), np.float32)

    jpad = np.arange(JPAD)
    jvalid = np.zeros(JPAD, bool)
    jvalid[:NUM_NULL] = True
    jvalid[NUM_NULL:NUM_NULL + N] = mask
    key_of_j = jpad - NUM_NULL

    in_maps = []
    idx_all = []
    for c in range(NCORES):
        idx = np.concatenate([np.arange(c, 1024, 8), np.arange(1024 + c, 2048, 8)])
        idx_all.append(idx)
        allow = jvalid[None, :] & (key_of_j[None, :] <= idx[:, None])  # [NQ, JPAD]
        allow[:, :NUM_NULL] = True
        ab = np.zeros((HEADS, JPAD, NQ), np.float32)
        ab[:, NUM_NULL:NUM_NULL + N, :] = attn_bias[:, idx, :].transpose(0, 2, 1)
        btraw = np.where(allow.T[None], ab, MASK_VAL)  # raw bias+mask
        bt = np.exp(btraw)                             # exp(bias), masked->0
        pk = np.empty((HEADS // 2, 128, EWP), np.float32)
        for jt in range(SH_JT):
            c0 = jt * 512
            tile = bt[:, jt * 128:(jt + 1) * 128, :]        # [H, 128, 256]
            pk[:, :, c0:c0 + 128] = tile[0::2, :, 0:128]         # h0 b0
            pk[:, :, c0 + 128:c0 + 256] = tile[1::2, :, 0:128]   # h1 b0
            pk[:, :, c0 + 256:c0 + 384] = tile[0::2, :, 128:256]  # h0 b1
            pk[:, :, c0 + 384:c0 + 512] = tile[1::2, :, 128:256]  # h1 b1
        for jt in range(SH_JT, JT):
            c0 = SH_JT * 512 + (jt - SH_JT) * 256
            # jt >= 13 (psum chunk 4): RAW additive bias, added on PE
            src = bt if jt < 13 else btraw
            tile = src[:, jt * 128:(jt + 1) * 128, 128:256]  # [H, 128, 128]
            pk[:, :, c0:c0 + 128] = tile[0::2]
            pk[:, :, c0 + 128:c0 + 256] = tile[1::2]
        in_maps.append({
            "xtq": np.ascontiguousarray(xt[:, idx]).astype(bfloat16),
            "xt": xt.astype(bfloat16),
            "wq": wq_eff.astype(bfloat16),
            "srow": srow.astype(bfloat16),
            "wkv": Wkv.astype(bfloat16),
            "nkvt": nkvt.astype(bfloat16),
            "wout": Wout.astype(bfloat16),
            "ibf": I128.astype(bfloat16),
            "onesbf": ones.astype(bfloat16),
            "onesf": ones,
            "ebias": pk.astype(bfloat16),
        })
    return in_maps, idx_all


def _run(inputs, trace=False):
    from concourse.bass_utils import run_bass_kernel_spmd
    if "nc" not in _CACHE:
        _CACHE["nc"] = _build_graph()
    nc = _CACHE["nc"]
    in_maps, idx_all = _prep_inputs(**inputs)
    res = run_bass_kernel_spmd(nc, in_maps, list(range(NCORES)), trace=trace)
    out = np.zeros((B, N, DIM), np.float32)
    for c in range(NCORES):
        out[0, idx_all[c], :] = res.results[c]["out"].T
    return out, res


def kernel(**inputs):
    out, _ = _run(inputs, trace=False)
    return out


# revision 58
# speedup vs baseline: 1.1509x; 1.0130x over previous
"""Trainium2 8-core kernel for nn_Attention_70892730187933 (sparse multi-query attention).

Sharding: sequence-parallel over query rows. Core c owns rows {i : i % 8 == c},
as 2 blocks of 128 rows (block0 < 1024, block1 >= 1024). Causal trimming:
block0 needs key j-tiles 0..8, block1 needs 0..16 (key space padded to
17*128 = 2176 incl. 2 null cols). No collectives; host concatenates rows.

Activations stay transposed [feature, token] so every matmul contraction dim
lands on partitions with no on-device activation transposes. LayerNorm stats
come from PE ones-column matmuls on bf16 x^T (raw moments); the mean
correction folds into the Q projection (rank-1 srow term). The bias+mask is
applied MULTIPLICATIVELY: host precomputes exp(bias+mask) (masked -> 0), and
after ScalarE computes exp(sim) from PSUM, DVE and Pool (alternating chunks)
multiply it in bf16 -- no per-tile identity bias matmuls on the Tensor
engine. PE stationary loads (LDWEIGHTS ~145ns each) are halved where tiles
repeat: stats/broadcast ones vectors, kv projection run ct-outer (one wkv
load serves 4 column chunks), and o-matmuls process TWO head-pairs jointly
(one V-tile load serves both pairs' E) via ldweights=False on the repeats.
Rowsums ride as a ones-column in V; per-pair SBUF->SBUF DMAs scatter them
into a [64, 64] tile laid out so a direct flatten DMA yields the even/odd-
head-permuted reciprocal row for 8 PE broadcast matmuls + 3 normalize mults.

Raw Block + explicit semaphores: this walrus build rejects instructions with
multiple attached sync waits, so Tile-generated sync cannot compile; every
cross-engine wait here is its own sequencer instruction. The builder plans all
five engine programs in one pass (semaphore counters known at plan time), then
emits them inside one Block.
"""

import sys
import numpy as np

sys.path.insert(0, "/opt/trn_rl_repo")

B, N, DIM, HEADS, DIM_HEAD, NUM_NULL = 1, 2048, 1024, 16, 64, 2
INNER = HEADS * DIM_HEAD
EPS = 1e-5
NCORES = 8
JT = 17
JPAD = JT * 128
NQ = 256
MASK_VAL = -30000.0
SH_JT = 9
NPAIR = HEADS // 2
EWP = SH_JT * 512 + (JT - SH_JT) * 256  # 6656: packed pair E/ebias width
# psum chunks: 5 per pair so two [65,512] o-accumulators fit alongside.
# A-region: ck0,ck2,ck4 (max 1536); B-region: ck1,ck3 (max 1536).
CH_OF_JT = [0, 0, 0, 1, 1, 1, 2, 2, 2, 3, 3, 3, 3, 4, 4, 4, 4]
CH_LEN = [1536, 1536, 1536, 1024, 1024]
CH_ECOL = [0, 1536, 3072, 4608, 5632]
CH_ENG = ["vector", "vector", "vector", "vector", "vector"]  # E-mult engine
NCH = 5

_CACHE = {}


def _ecol(jt):
    """Column of j-tile jt in packed pair E/ebias layout."""
    return jt * 512 if jt < SH_JT else SH_JT * 512 + (jt - SH_JT) * 256


def _ccol(jt):
    """Column of j-tile jt inside its psum chunk."""
    if jt < SH_JT:
        return (jt % 3) * 512
    if jt < 13:
        return (jt - 9) * 256
    return (jt - 13) * 256


def _build_graph():
    from contextlib import ExitStack
    import concourse.bass as bass
    import concourse.mybir as mybir

    dt = mybir.dt
    F32, BF16 = dt.float32, dt.bfloat16
    AF = mybir.ActivationFunctionType
    OP = mybir.AluOpType
    nc = bass.Bass()

    d_in = {}
    for name, shape, ty in [
        ("xtq", [DIM, NQ], BF16), ("xt", [DIM, N], BF16),
        ("wq", [DIM, INNER], BF16), ("srow", [1, INNER], BF16),
        ("wkv", [DIM, 2 * DIM_HEAD], BF16), ("nkvt", [128, NUM_NULL], BF16),
        ("wout", [INNER, DIM], BF16), ("ibf", [128, 128], BF16),
        ("onesbf", [1, 128], BF16), ("onesf", [1, 128], F32),
        ("ebias", [NPAIR, 128, EWP], BF16),
    ]:
        d_in[name] = nc.declare_dram_parameter(name, shape, ty, isOutput=False)
    out_d = nc.declare_dram_parameter("out", [DIM, NQ], F32, isOutput=True)

    ctx = ExitStack()
    sb = {}
    for name, shape, ty in [
        ("ibf", [128, 128], BF16), ("onesbf", [1, 128], BF16),
        ("onesf", [1, 128], F32), ("srow", [1, INNER], BF16),
        ("zb", [128, 1], F32), ("epsb", [128, 1], F32),
        ("onescol", [128, 1], BF16),
        ("wq", [128, 8 * INNER], BF16), ("wout", [128, 8 * DIM], BF16),
        ("wkv", [128, 8 * 128], BF16), ("xt", [128, 8 * N], BF16),
        ("xtq", [128, 8 * NQ], BF16), ("xsq", [128, 8 * NQ], BF16),
        ("lnrow", [1, 1024], F32),  # 0:256 negmu, 256:512 ex2, 512:768 var, 768:1024 rsq
        ("negmurs", [1, NQ], BF16),
        ("xst", [128, 8 * NQ], BF16), ("qtmp", [128, 2 * NQ], BF16),
        ("kv", [128, JPAD], BF16), ("vsb", [128, JT * 65], BF16),
        ("e0", [128, EWP], BF16), ("e1", [128, EWP], BF16),
        ("b0", [128, EWP], BF16), ("b1", [128, EWP], BF16),
        ("oT", [64, HEADS * NQ], BF16), ("rs_row", [65, 2 * 512], F32),
        ("rs64", [64, 64], F32), ("r64f", [64, 64], F32), ("r64b", [64, 64], BF16),
        ("recipflat", [1, HEADS * NQ], BF16),
        ("oTn", [128, 8 * NQ], BF16), ("oTn_lo", [64, 8 * NQ], BF16),
        ("outsb", [128, 2 * NQ], F32),
    ] + [(f"qh{h}", [64, 2 * NQ], BF16) for h in range(NPAIR)]:
        sb[name] = ctx.enter_context(nc.sbuf_tensor("sb_" + name, shape, ty))

    qh = [sb[f"qh{h}"] for h in range(NPAIR)]
    esb = [sb["e0"], sb["e1"]]
    bsb = [sb["b0"], sb["b1"]]

    # PSUM: early tensors freed before head-loop tensors are allocated.
    early = ExitStack()
    kvp = early.enter_context(nc.psum_tensor("kvp", [128, 1024], F32))
    qp = [early.enter_context(nc.psum_tensor(f"qp{i}", [128, NQ], F32))
          for i in range(2)]
    vp = [early.enter_context(nc.psum_tensor(f"vp{i}", [128, 64], BF16))
          for i in range(2)]
    stp2 = early.enter_context(nc.psum_tensor("stp2", [1, 512], F32))
    rbp = early.enter_context(nc.psum_tensor("rbp", [128, NQ], F32))
    early.close()
    simA = ctx.enter_context(nc.psum_tensor("simA", [128, 1536], F32))
    simB = ctx.enter_context(nc.psum_tensor("simB", [128, 1536], F32))
    opp2 = ctx.enter_context(nc.psum_tensor("opp2", [65, 1024], F32))

    # ------- planner -------
    plan = {"sync": [], "tensor": [], "vector": [], "scalar": [], "gpsimd": []}
    DSEMS = ("dk", "dxq", "dqx", "dsm", "dw",
             "db0", "db1", "dq", "dr", "dm", "dn", "do")
    cnt = {"p": 0, "v": 0, "s": 0, "g": 0, **{k: 0 for k in DSEMS}}
    SEM = {}
    ESEM = {"vector": "v", "gpsimd": "g", "scalar": "s"}

    def wait(eng, sem, thr):
        if thr > 0:
            plan[eng].append(lambda e, s=sem, t=thr: e.wait_ge(SEM[s], t))

    def dma(sem, out, in_, eng="sync"):
        cnt[sem] += 16
        plan[eng].append(
            lambda e, s=sem, o=out, i=in_: e.dma_start(out=o, in_=i)
            .then_inc(SEM[s], 16))
        return cnt[sem]

    def inc(eng, sem, fn):
        # DVE/ACT/Pool pipelines expose writes only after a drain; put the sem
        # update on the drain so consumers see committed data. Scalar ops
        # ALSO fire "sr" on the op itself (inputs consumed -> psum reusable),
        # keeping sr/s counts in lockstep.
        cnt[sem] += 1
        if eng == "scalar" and sem == "s":
            plan[eng].append(lambda e, f=fn: f(e).then_inc(SEM["sr"], 1))
            plan[eng].append(lambda e, s=sem: e.drain().then_inc(SEM[s], 1))
        elif eng in ("vector", "scalar", "gpsimd"):
            plan[eng].append(lambda e, f=fn: f(e))
            plan[eng].append(lambda e, s=sem: e.drain().then_inc(SEM[s], 1))
        else:
            plan[eng].append(lambda e, f=fn, s=sem: f(e).then_inc(SEM[s], 1))
        return cnt[sem]

    def run(eng, fn):
        plan[eng].append(fn)

    def runD(eng, fn):
        # run + drain, for same-engine RAW chains without a semaphore inc
        plan[eng].append(fn)
        plan[eng].append(lambda e: e.drain())

    def mm_reuse(e, *args, **kwargs):
        # matmul whose stationary was loaded by the immediately preceding
        # matmul on PE: suppress this instruction's LDWEIGHTS.
        m = e.matmul(*args, **kwargs)
        m.ins.ldweights = False
        return m

    # ========== DMA issue order (sync queue = priority order) ==========
    def dma8(sem, nm, eng="sync"):
        return dma(sem, sb[nm][:].rearrange("p (c f) -> p c f", c=8),
                   d_in[nm][:].rearrange("(c p) f -> p c f", c=8), eng=eng)

    d_xq = dma8("dxq", "xtq")
    dma8("dqx", "wq")
    d_qx = cnt["dqx"]
    for nm in ("ibf", "onesbf", "onesf", "srow"):
        dma("dsm", sb[nm][:], d_in[nm][:])
    d_sm0 = dma("dsm", sb["kv"][:, 0:NUM_NULL], d_in["nkvt"][:])
    dma8("dk", "wkv")
    dma8("dk", "xt")
    d_kv = cnt["dk"]
    for p in range(2):
        dma(f"db{p}", bsb[p][:], d_in["ebias"][p])
    d_w = dma8("dw", "wout")  # after ebias: not needed until the tail

    # ========== VECTOR: memsets ==========
    run("vector", lambda e: e.memset(sb["zb"][:], 0.0))
    run("vector", lambda e: e.memset(sb["epsb"][:], EPS))
    run("vector", lambda e: e.memset(sb["onescol"][:], 1.0))
    run("vector", lambda e: e.memset(sb["vsb"][:], 1.0))
    v_memset = inc("vector", "v",
                   lambda e: e.memset(sb["kv"][:, NUM_NULL + N:JPAD], 0.0))

    # ========== LN stats (row layout, raw moments) ==========
    wait("scalar", "dxq", d_xq)
    wait("scalar", "v", v_memset)  # zb ready
    s_xsq = inc("scalar", "s", lambda e: e.activation(
        out=sb["xsq"][:], in_=sb["xtq"][:], func=AF.Square, bias=sb["zb"][:]))

    # tensor: musum (cols 0:256) and sqsum (cols 256:512) into stp2;
    # all 16 matmuls share the onescol stationary (one LDWEIGHTS).
    wait("tensor", "dxq", d_xq)
    wait("tensor", "v", v_memset)  # onescol ready
    for ct in range(8):
        def fn(e, ct=ct):
            f = e.matmul if ct == 0 else (lambda *a, **k: mm_reuse(e, *a, **k))
            return f(stp2[0:1, 0:NQ], sb["onescol"][:],
                     sb["xtq"][:, ct * NQ:(ct + 1) * NQ],
                     start=(ct == 0), stop=(ct == 7))
        if ct == 7:
            p_mu = inc("tensor", "p", fn)
        else:
            run("tensor", fn)

    wait("tensor", "s", s_xsq)
    for ct in range(8):
        fn = lambda e, ct=ct: mm_reuse(
            e, stp2[0:1, NQ:2 * NQ], sb["onescol"][:],
            sb["xsq"][:, ct * NQ:(ct + 1) * NQ],
            start=(ct == 0), stop=(ct == 7))
        if ct == 7:
            p_sq = inc("tensor", "p", fn)
        else:
            run("tensor", fn)

    # vector rowops (RAW chains -> drained)
    neg_mu = sb["lnrow"][0:1, 0:256]
    ex2 = sb["lnrow"][0:1, 256:512]
    var_r = sb["lnrow"][0:1, 512:768]
    rsq_r = sb["lnrow"][0:1, 768:1024]
    wait("vector", "p", p_mu)
    runD("vector", lambda e: e.tensor_scalar_mul(
        out=neg_mu, in0=stp2[0:1, 0:NQ], scalar1=-1.0 / DIM))
    wait("vector", "p", p_sq)
    runD("vector", lambda e: e.tensor_scalar_mul(
        out=ex2, in0=stp2[0:1, NQ:2 * NQ], scalar1=1.0 / DIM))
    runD("vector", lambda e: e.tensor_tensor(
        out=var_r, in0=neg_mu, in1=neg_mu, op=OP.mult))
    v_var = inc("vector", "v", lambda e: e.tensor_tensor(
        out=var_r, in0=ex2, in1=var_r, op=OP.subtract))
    # scalar: rsq = exp(-0.5 * ln(var + eps))
    wait("scalar", "v", v_var)
    runD("scalar", lambda e: e.activation(
        out=var_r, in_=var_r, func=AF.Ln, scale=1.0, bias=sb["epsb"][0:1, :]))
    s_rsq = inc("scalar", "s", lambda e: e.activation(
        out=rsq_r, in_=var_r, func=AF.Exp, scale=-0.5, bias=sb["zb"][0:1, :]))
    wait("vector", "s", s_rsq)
    v_nmr = inc("vector", "v", lambda e: e.tensor_tensor(
        out=sb["negmurs"][:], in0=neg_mu, in1=rsq_r, op=OP.mult))

    # tensor: rsqb broadcast [128, 256] (f32 matmul)
    wait("tensor", "dsm", d_sm0)  # onesf (+ibf/srow/nkvt) loaded
    wait("tensor", "s", s_rsq)
    p_rsqb = inc("tensor", "p", lambda e: e.matmul(
        rbp[:], sb["onesf"][0:1, :], rsq_r, start=True, stop=True))

    # vector: xst = xtq * rsqb
    wait("vector", "p", p_rsqb)
    for ct in range(8):
        fn = lambda e, ct=ct: e.tensor_tensor(
            out=sb["xst"][:, ct * NQ:(ct + 1) * NQ],
            in0=sb["xtq"][:, ct * NQ:(ct + 1) * NQ], in1=rbp[:], op=OP.mult)
        if ct == 7:
            v_xst = inc("vector", "v", fn)
        else:
            run("vector", fn)

    # ========== TENSOR: q projection (qp double-buffered) ==========
    wait("tensor", "v", v_xst)
    wait("tensor", "dqx", d_qx)
    wait("tensor", "v", v_nmr)
    p_q = [0] * 8
    v_qtmp = [0] * 8
    for dtile in range(8):
        pb = qp[dtile % 2]
        if dtile >= 2:
            wait("tensor", "v", v_qtmp[dtile - 2])
        for ct in range(8):
            run("tensor", lambda e, pb=pb, dtile=dtile, ct=ct: e.matmul(
                pb[:],
                sb["wq"][:, ct * INNER + dtile * 128:ct * INNER + (dtile + 1) * 128],
                sb["xst"][:, ct * NQ:(ct + 1) * NQ],
                start=(ct == 0), stop=False))
        p_q[dtile] = inc("tensor", "p", lambda e, pb=pb, dtile=dtile: e.matmul(
            pb[:], sb["srow"][0:1, dtile * 128:(dtile + 1) * 128],
            sb["negmurs"][:], start=False, stop=True))
        wait("vector", "p", p_q[dtile])
        run("vector", lambda e, pb=pb, dtile=dtile: e.tensor_copy(
            qh[dtile][0:64, :].rearrange("a (b h c) -> a b h c", b=2, h=2)[:, :, 0, :],
            pb[0:64, :].rearrange("a (b c) -> a b c", b=2)))
        slot = dtile % 2
        if dtile >= 2:
            wait("vector", "dq", 16 * dtile)  # qtmp slot reuse (all issued)
        v_qtmp[dtile] = inc("vector", "v", lambda e, pb=pb, slot=slot:
                            e.tensor_copy(
                                sb["qtmp"][64:128, slot * NQ:(slot + 1) * NQ],
                                pb[64:128, :]))
        wait("sync", "v", v_qtmp[dtile])
        dma("dq",
            qh[dtile][0:64, :].rearrange("a (b h c) -> a b h c", b=2, h=2)[:, :, 1, :],
            sb["qtmp"][64:128, slot * NQ:(slot + 1) * NQ]
            .rearrange("a (b c) -> a b c", b=2))

    # ========== TENSOR: kv matmuls, ct-outer (one wkv load per ct) ==========
    wait("tensor", "dk", d_kv)
    s_kvevac = [0] * 4
    p_kvh = [0, 0]
    for half in range(2):
        if half == 1:
            wait("tensor", "s", s_kvevac[1])  # kvp reuse
        for ct in range(8):
            for chh in range(2):
                ch = half * 2 + chh
                def fn(e, ct=ct, ch=ch, chh=chh):
                    f = e.matmul if chh == 0 else (
                        lambda *a, **k: mm_reuse(e, *a, **k))
                    return f(
                        kvp[:, chh * 512:(chh + 1) * 512],
                        sb["wkv"][:, ct * 128:(ct + 1) * 128],
                        sb["xt"][:, ct * N + ch * 512:ct * N + (ch + 1) * 512],
                        start=(ct == 0), stop=(ct == 7))
                if ct == 7 and chh == 1:
                    p_kvh[half] = inc("tensor", "p", fn)
                else:
                    run("tensor", fn)
        wait("scalar", "p", p_kvh[half])
        for chh in range(2):
            ch = half * 2 + chh
            s_kvevac[ch] = inc("scalar", "s", lambda e, ch=ch, chh=chh:
                               e.activation(
                                   out=sb["kv"][:, NUM_NULL + ch * 512:
                                                NUM_NULL + (ch + 1) * 512],
                                   in_=kvp[:, chh * 512:(chh + 1) * 512],
                                   func=AF.Copy))

    # ========== TENSOR: v transposes (vp double-buffered) ==========
    p_vt = [0] * JT
    v_vcopy = [0] * JT
    for jt in range(JT):
        pb = vp[jt % 2]
        ch_hi = min(3, ((jt + 1) * 128 - 1 - NUM_NULL) // 512)
        wait("tensor", "s", s_kvevac[ch_hi])
        if jt == JT - 1:
            wait("tensor", "v", v_memset)
        if jt >= 2:
            wait("tensor", "v", v_vcopy[jt - 2])
        p_vt[jt] = inc("tensor", "p", lambda e, pb=pb, jt=jt: e.transpose(
            pb[:], sb["kv"][64:128, jt * 128:(jt + 1) * 128],
            sb["ibf"][64:128, 64:128]))
        wait("vector", "p", p_vt[jt])
        v_vcopy[jt] = inc("vector", "v", lambda e, pb=pb, jt=jt: e.tensor_copy(
            sb["vsb"][:, jt * 65:jt * 65 + 64], pb[:]))
    v_vsb = v_vcopy[JT - 1]

    # ========== PAIR LOOP: sims per pair, o-matmuls joint per pair-pair ====
    v_pre_heads = cnt["v"]
    s_exp = [[0] * NCH for _ in range(NPAIR)]
    e_mult = [[None] * NCH for _ in range(NPAIR)]  # (sem_name, count)
    p_simc = [[0] * NCH for _ in range(NPAIR)]
    p_odone = [0] * (NPAIR // 2)
    v_oevac = [0] * (NPAIR // 2)

    wait("tensor", "s", s_kvevac[3])
    wait("tensor", "v", v_pre_heads)  # early-psum reuse guard
    wait("tensor", "dq", 16 * 8)      # all odd-half q DMAs done

    def emit_pair_sims(p):
        """sims + exp + E-mult for pair p (5 chunks)."""
        eh = esb[p % 2]
        bh = bsb[p % 2]
        qpair = qh[p][0:64, :]
        qsolo = qh[p][0:64, NQ:2 * NQ]
        for ck in range(NCH):
            ps = simA if ck in (0, 2, 4) else simB
            jts = [jt for jt in range(JT) if CH_OF_JT[jt] == ck]
            # psum region reuse: wait on exp-READ of the previous occupant
            if ck in (0, 1):
                if p >= 1:
                    wait("tensor", "sr", s_exp[p - 1][{0: 4, 1: 3}[ck]])
            else:
                wait("tensor", "sr", s_exp[p][ck - 2])
            for jt in jts:
                w = 512 if jt < SH_JT else 256
                rhs = qpair if jt < SH_JT else qsolo
                fn = lambda e, ps=ps, jt=jt, r=rhs, w=w: e.matmul(
                    ps[:, _ccol(jt):_ccol(jt) + w],
                    sb["kv"][0:64, jt * 128:(jt + 1) * 128], r,
                    start=True, stop=True)
                if jt == jts[-1]:
                    p_simc[p][ck] = inc("tensor", "p", fn)
                else:
                    run("tensor", fn)
            # SCALAR: exp for this chunk. "sr" fires on the ACTIVATE itself
            # (psum READ done -> region reusable ~1us before the drain);
            # "s" fires on the drain (E write visible to mult/o consumers).
            wait("scalar", "p", p_simc[p][ck])
            if ck == 0 and p >= 2:
                wait("scalar", "p", p_odone[p // 2 - 1])  # E slot reuse
            ln = CH_LEN[ck]
            s_exp[p][ck] = inc("scalar", "s", lambda e, ps=ps, ck=ck, ln=ln,
                               eh=eh: e.activation(
                                   out=eh[:, CH_ECOL[ck]:CH_ECOL[ck] + ln],
                                   in_=ps[:, 0:ln], func=AF.Exp, bias=sb["zb"][:]))
            # DVE/Pool: E *= exp(bias) in place (bf16, all-SBUF)
            eng = CH_ENG[ck]
            sem = ESEM[eng]
            if ck in (0, 1):  # first chunk on each engine: ebias slot loaded
                wait(eng, f"db{p % 2}", 16 * (p // 2 + 1))
            wait(eng, "s", s_exp[p][ck])
            n = inc(eng, sem, lambda e, ck=ck, ln=ln, eh=eh, bh=bh:
                    e.tensor_tensor(
                        out=eh[:, CH_ECOL[ck]:CH_ECOL[ck] + ln],
                        in0=eh[:, CH_ECOL[ck]:CH_ECOL[ck] + ln],
                        in1=bh[:, CH_ECOL[ck]:CH_ECOL[ck] + ln],
                        op=OP.mult))
            e_mult[p][ck] = (sem, n)

    for pp in range(NPAIR // 2):
        pa, pb_ = 2 * pp, 2 * pp + 1
        emit_pair_sims(pa)
        emit_pair_sims(pb_)

        # TENSOR: joint o-matmuls for pairs (pa, pb_): per jt one V-tile
        # load serves both pairs (second matmul reuses the stationary).
        if pp == 0:
            wait("tensor", "v", v_vsb)
        if pp >= 1:
            wait("tensor", "v", v_oevac[pp - 1])  # opp2 reuse
        ea, eb_ = esb[0], esb[1]
        for jt in range(JT):
            ck = CH_OF_JT[jt]
            if jt == 0 or _ccol(jt) == 0:
                for pq in (pa, pb_):
                    sem, n = e_mult[pq][ck]
                    wait("tensor", sem, n)
            w, eoff = (512, 0) if jt < SH_JT else (256, 256)
            fn_a = lambda e, jt=jt, w=w, eoff=eoff: e.matmul(
                opp2[0:65, eoff:eoff + w],
                sb["vsb"][:, jt * 65:jt * 65 + 65],
                ea[:, _ecol(jt):_ecol(jt) + w],
                start=(jt == 0), stop=(jt == JT - 1))
            fn_b = lambda e, jt=jt, w=w, eoff=eoff: mm_reuse(
                e, opp2[0:65, 512 + eoff:512 + eoff + w],
                sb["vsb"][:, jt * 65:jt * 65 + 65],
                eb_[:, _ecol(jt):_ecol(jt) + w],
                start=(jt == 0), stop=(jt == JT - 1))
            run("tensor", fn_a)
            if jt == JT - 1:
                p_odone[pp] = inc("tensor", "p", fn_b)
            else:
                run("tensor", fn_b)

        # VECTOR: evacuate o rows + rowsum rows for both pairs
        wait("vector", "p", p_odone[pp])
        if pp >= 1:
            wait("vector", "dr", 64 * pp)  # rs_row slots reuse (all issued)
        for si, pq in enumerate((pa, pb_)):
            run("vector", lambda e, si=si, pq=pq: e.tensor_copy(
                sb["oT"][0:64, pq * 512:(pq + 1) * 512]
                .rearrange("a (h b c) -> a h b c", h=2, b=2),
                opp2[0:64, si * 512:(si + 1) * 512]
                .rearrange("a (b h c) -> a h b c", b=2, h=2)))
        for si, pq in enumerate((pa, pb_)):
            fn = lambda e, si=si: e.tensor_copy(
                sb["rs_row"][64:65, si * 512:(si + 1) * 512]
                .rearrange("a (h b c) -> a h b c", h=2, b=2),
                opp2[64:65, si * 512:(si + 1) * 512]
                .rearrange("a (b h c) -> a h b c", b=2, h=2))
            if si == 1:
                v_oevac[pp] = inc("vector", "v", fn)
            else:
                run("vector", fn)
        # SYNC: scatter rowsums into rs64. Layout: partition q = h*32 + p*4
        # + bc//64, col = bc%64 -- partition-major iteration of rs64 is then
        # exactly the even/odd-head-permuted recipflat order.
        wait("sync", "v", v_oevac[pp])
        for si, pq in enumerate((pa, pb_)):
            for h in range(2):
                dma("dr", sb["rs64"][h * 32 + pq * 4:h * 32 + pq * 4 + 4, :],
                    sb["rs_row"][64:65,
                                 si * 512 + h * 256:si * 512 + (h + 1) * 256])

        # SYNC: ebias prefetch for pairs pa+2, pb_+2 (slot free once both
        # engines' E-mults for the current occupant finish)
        for pq in (pa, pb_):
            if pq + 2 < NPAIR:
                for s in ("v", "g"):
                    ent = [e_mult[pq][c] for c in range(NCH)
                           if e_mult[pq][c][0] == s]
                    if ent:
                        wait("sync", s, ent[-1][1])
                dma(f"db{pq % 2}", bsb[pq % 2][:], d_in["ebias"][pq + 2])

    # ========== tail: reciprocal + normalize + output projection ==========
    wait("vector", "dr", cnt["dr"])
    runD("vector", lambda e: e.reciprocal(out=sb["r64f"][:], in_=sb["rs64"][:]))
    v_recip = inc("vector", "v", lambda e: e.tensor_copy(
        sb["r64b"][:], sb["r64f"][:]))
    # direct SBUF->SBUF flatten: partition-major r64b == permuted order
    wait("sync", "v", v_recip)
    d_m = dma("dm", sb["recipflat"][0:1, :], sb["r64b"][:])

    # PE: broadcast matmuls R = ones64 x recipflat (one LDWEIGHTS total)
    wait("tensor", "dm", d_m)
    wait("tensor", "sr", s_exp[NPAIR - 1][NCH - 1])  # simA/simB read-free
    wait("tensor", "v", v_oevac[NPAIR // 2 - 1])     # opp2 free
    p_bc = [0] * 3
    # R_even (heads 0,2,..14) -> simB (3 x 512) + opp2[0:64, 0:512]
    # R_odd  (heads 1,3,..15) -> simA (3 x 512) + opp2[0:64, 512:1024]
    bc_dsts = ([(simB, i * 512, i * 512) for i in range(3)]
               + [(opp2, 0, 1536)]
               + [(simA, i * 512, 2048 + i * 512) for i in range(3)]
               + [(opp2, 512, 3584)])
    for i, (ps, poff, roff) in enumerate(bc_dsts):
        fn = lambda e, i=i, ps=ps, poff=poff, roff=roff: (
            e.matmul if i == 0 else (lambda *a, **k: mm_reuse(e, *a, **k)))(
            ps[0:64, poff:poff + 512], sb["onesbf"][0:1, 0:64],
            sb["recipflat"][0:1, roff:roff + 512], start=True, stop=True)
        if i in (3, 6, 7):
            p_bc[{3: 0, 6: 1, 7: 2}[i]] = inc("tensor", "p", fn)
        else:
            run("tensor", fn)

    # VECTOR: oTn = oT * R  (even heads -> oTn[0:64], odd -> oTn_lo)
    oT_hp = sb["oT"][0:64, :].rearrange("a (p k c) -> a p k c", p=8, k=2)
    wait("vector", "p", p_bc[0])
    v_n0 = inc("vector", "v", lambda e: e.tensor_tensor(
        out=sb["oTn"][0:64, 0:1536].rearrange("a (p c) -> a p c", p=6),
        in0=oT_hp[:, 0:6, 0, :],
        in1=simB[0:64, 0:1536].rearrange("a (p c) -> a p c", p=6),
        op=OP.mult))
    run("vector", lambda e: e.tensor_tensor(
        out=sb["oTn"][0:64, 1536:2048].rearrange("a (p c) -> a p c", p=2),
        in0=oT_hp[:, 6:8, 0, :],
        in1=opp2[0:64, 0:512].rearrange("a (p c) -> a p c", p=2),
        op=OP.mult))
    wait("vector", "p", p_bc[1])
    run("vector", lambda e: e.tensor_tensor(
        out=sb["oTn_lo"][0:64, 0:1536].rearrange("a (p c) -> a p c", p=6),
        in0=oT_hp[:, 0:6, 1, :],
        in1=simA[0:64, 0:1536].rearrange("a (p c) -> a p c", p=6),
        op=OP.mult))
    wait("vector", "p", p_bc[2])
    v_n2 = inc("vector", "v", lambda e: e.tensor_tensor(
        out=sb["oTn_lo"][0:64, 1536:2048].rearrange("a (p c) -> a p c", p=2),
        in0=oT_hp[:, 6:8, 1, :],
        in1=opp2[0:64, 512:1024].rearrange("a (p c) -> a p c", p=2),
        op=OP.mult))
    wait("sync", "v", v_n2)
    d_n = dma("dn", sb["oTn"][64:128, :], sb["oTn_lo"][0:64, :])

    # ========== output projection (accumulators alternate in simB banks) ====
    wait("tensor", "v", v_n0)
    wait("tensor", "dn", d_n)
    wait("tensor", "dw", d_w)
    s_outevac = [0] * 8
    for et in range(8):
        reg = simB[:, 0:NQ] if et % 2 == 0 else simB[:, 512:512 + NQ]
        if et >= 2:
            wait("tensor", "s", s_outevac[et - 2])
        p_wout = 0
        for hdt in range(8):
            fn = lambda e, et=et, hdt=hdt, reg=reg: e.matmul(
                reg, sb["wout"][:, hdt * DIM + et * 128:hdt * DIM + (et + 1) * 128],
                sb["oTn"][:, hdt * NQ:(hdt + 1) * NQ],
                start=(hdt == 0), stop=(hdt == 7))
            if hdt == 7:
                p_wout = inc("tensor", "p", fn)
            else:
                run("tensor", fn)
        wait("scalar", "p", p_wout)
        oslot = et % 2
        if et >= 2:
            wait("scalar", "do", 16 * et)  # outsb slot reuse (all issued)
        s_outevac[et] = inc("scalar", "s", lambda e, oslot=oslot, reg=reg:
                            e.activation(
                                out=sb["outsb"][:, oslot * NQ:(oslot + 1) * NQ],
                                in_=reg, func=AF.Copy))
        wait("sync", "s", s_outevac[et])
        dma("do", out_d[et * 128:(et + 1) * 128, :],
            sb["outsb"][:, oslot * NQ:(oslot + 1) * NQ])

    # ========== emit ==========
    from contextlib import ExitStack as _ES
    semctx = _ES()
    for k in ("p", "v", "s", "sr", "g") + DSEMS:
        SEM[k] = semctx.enter_context(nc.semaphore(f"sem_{k}"))
    with semctx:
        with nc.Block() as block:
            @block.sync
            def _(e):
                for fn in plan["sync"]:
                    fn(e)

            @block.tensor
            def _(e):
                for fn in plan["tensor"]:
                    fn(e)

            @block.vector
            def _(e):
                for fn in plan["vector"]:
                    fn(e)

            @block.scalar
            def _(e):
                for fn in plan["scalar"]:
                    fn(e)

            @block.gpsimd
            def _(e):
                for fn in plan["gpsimd"]:
                    fn(e)
    ctx.close()
    return nc


def _prep_inputs(x, attn_bias, Wq, Wkv, null_kv, Wout, gamma, mask):
    from ml_dtypes import bfloat16
    x = np.asarray(x, np.float32)[0]            # [N, DIM]
    attn_bias = np.asarray(attn_bias, np.float32)[0]  # [H, N, N]
    Wq = np.asarray(Wq, np.float32)
    Wkv = np.asarray(Wkv, np.float32)
    null_kv = np.asarray(null_kv, np.float32)
    Wout = np.asarray(Wout, np.float32)
    gamma = np.asarray(gamma, np.float32)
    mask = np.asarray(mask, bool)[0]            # [N]

    scale = DIM_HEAD ** -0.5
    wq_eff = (gamma[:, None] * Wq * scale).astype(np.float32)
    srow = wq_eff.sum(axis=0, keepdims=True)
    xt = np.ascontiguousarray(x.T)
    nkvt = np.zeros((128, NUM_NULL), np.float32)
    nkvt[0:DIM_HEAD, :] = null_kv[0].T
    nkvt[64:64 + DIM_HEAD, :] = null_kv[1].T
    I128 = np.eye(128, dtype=np.float32)
    ones = np.ones((1, 128), np.float32)

    jpad = np.arange(JPAD)
    jvalid = np.zeros(JPAD, bool)
    jvalid[:NUM_NULL] = True
    jvalid[NUM_NULL:NUM_NULL + N] = mask
    key_of_j = jpad - NUM_NULL

    in_maps = []
    idx_all = []
    for c in range(NCORES):
        idx = np.concatenate([np.arange(c, 1024, 8), np.arange(1024 + c, 2048, 8)])
        idx_all.append(idx)
        allow = jvalid[None, :] & (key_of_j[None, :] <= idx[:, None])  # [NQ, JPAD]
        allow[:, :NUM_NULL] = True
        ab = np.zeros((HEADS, JPAD, NQ), np.float32)
        ab[:, NUM_NULL:NUM_NULL + N, :] = attn_bias[:, idx, :].transpose(0, 2, 1)
        bt = np.exp(np.where(allow.T[None], ab, MASK_VAL))  # exp(bias), masked->0
        pk = np.empty((HEADS // 2, 128, EWP), np.float32)
        for jt in range(SH_JT):
            c0 = jt * 512
            tile = bt[:, jt * 128:(jt + 1) * 128, :]        # [H, 128, 256]
            pk[:, :, c0:c0 + 128] = tile[0::2, :, 0:128]         # h0 b0
            pk[:, :, c0 + 128:c0 + 256] = tile[1::2, :, 0:128]   # h1 b0
            pk[:, :, c0 + 256:c0 + 384] = tile[0::2, :, 128:256]  # h0 b1
            pk[:, :, c0 + 384:c0 + 512] = tile[1::2, :, 128:256]  # h1 b1
        for jt in range(SH_JT, JT):
            c0 = SH_JT * 512 + (jt - SH_JT) * 256
            tile = bt[:, jt * 128:(jt + 1) * 128, 128:256]  # [H, 128, 128]
            pk[:, :, c0:c0 + 128] = tile[0::2]
            pk[:, :, c0 + 128:c0 + 256] = tile[1::2]
        in_maps.append({
            "xtq": np.ascontiguousarray(xt[:, idx]).astype(bfloat16),
            "xt": xt.astype(bfloat16),
            "wq": wq_eff.astype(bfloat16),
            "srow": srow.astype(bfloat16),
            "wkv": Wkv.astype(bfloat16),
            "nkvt": nkvt.astype(bfloat16),
            "wout": Wout.astype(bfloat16),
            "ibf": I128.astype(bfloat16),
            "onesbf": ones.astype(bfloat16),
            "onesf": ones,
            "ebias": pk.astype(bfloat16),
        })
    return in_maps, idx_all


def _run(inputs, trace=False):
    from concourse.bass_utils import run_bass_kernel_spmd
    if "nc" not in _CACHE:
        _CACHE["nc"] = _build_graph()
    nc = _CACHE["nc"]
    in_maps, idx_all = _prep_inputs(**inputs)
    res = run_bass_kernel_spmd(nc, in_maps, list(range(NCORES)), trace=trace)
    out = np.zeros((B, N, DIM), np.float32)
    for c in range(NCORES):
        out[0, idx_all[c], :] = res.results[c]["out"].T
    return out, res


def kernel(**inputs):
    out, _ = _run(inputs, trace=False)
    return out


# revision 59
# speedup vs baseline: 1.1760x; 1.0218x over previous
"""Trainium2 8-core kernel for nn_Attention_70892730187933 (sparse multi-query attention).

Sharding: sequence-parallel over query rows. Core c owns rows {i : i % 8 == c},
as 2 blocks of 128 rows (block0 < 1024, block1 >= 1024). Causal trimming:
block0 needs key j-tiles 0..8, block1 needs 0..16 (key space padded to
17*128 = 2176 incl. 2 null cols). No collectives; host concatenates rows.

Activations stay transposed [feature, token] so every matmul contraction dim
lands on partitions with no on-device activation transposes. LayerNorm stats
come from PE ones-column matmuls on bf16 x^T (raw moments); the mean
correction folds into the Q projection (rank-1 srow term). The bias+mask is
applied MULTIPLICATIVELY: host precomputes exp(bias+mask) (masked -> 0), and
after ScalarE computes exp(sim) from PSUM, DVE and Pool (alternating chunks)
multiply it in bf16 -- no per-tile identity bias matmuls on the Tensor
engine. PE stationary loads (LDWEIGHTS ~145ns each) are halved where tiles
repeat: stats/broadcast ones vectors, kv projection run ct-outer (one wkv
load serves 4 column chunks), and o-matmuls process TWO head-pairs jointly
(one V-tile load serves both pairs' E) via ldweights=False on the repeats.
Rowsums ride as a ones-column in V; per-pair SBUF->SBUF DMAs scatter them
into a [64, 64] tile laid out so a direct flatten DMA yields the even/odd-
head-permuted reciprocal row for 8 PE broadcast matmuls + 3 normalize mults.

Raw Block + explicit semaphores: this walrus build rejects instructions with
multiple attached sync waits, so Tile-generated sync cannot compile; every
cross-engine wait here is its own sequencer instruction. The builder plans all
five engine programs in one pass (semaphore counters known at plan time), then
emits them inside one Block.
"""

import sys
import numpy as np

sys.path.insert(0, "/opt/trn_rl_repo")

B, N, DIM, HEADS, DIM_HEAD, NUM_NULL = 1, 2048, 1024, 16, 64, 2
INNER = HEADS * DIM_HEAD
EPS = 1e-5
NCORES = 8
JT = 17
JPAD = JT * 128
NQ = 256
MASK_VAL = -30000.0
SH_JT = 9
NPAIR = HEADS // 2
EWP = SH_JT * 512 + (JT - SH_JT) * 256  # 6656: packed pair E/ebias width
# psum chunks: 5 per pair so two [65,512] o-accumulators fit alongside.
# A-region: ck0,ck2,ck4 (max 1536); B-region: ck1,ck3 (max 1536).
CH_OF_JT = [0, 0, 0, 1, 1, 1, 2, 2, 2, 3, 3, 3, 3, 4, 4, 4, 4]
CH_LEN = [1536, 1536, 1536, 1024, 1024]
CH_ECOL = [0, 1536, 3072, 4608, 5632]
CH_ENG = ["vector", "vector", "vector", "vector", "vector"]  # E-mult engine
NCH = 5

_CACHE = {}


def _ecol(jt):
    """Column of j-tile jt in packed pair E/ebias layout."""
    return jt * 512 if jt < SH_JT else SH_JT * 512 + (jt - SH_JT) * 256


def _ccol(jt):
    """Column of j-tile jt inside its psum chunk."""
    if jt < SH_JT:
        return (jt % 3) * 512
    if jt < 13:
        return (jt - 9) * 256
    return (jt - 13) * 256


def _build_graph():
    from contextlib import ExitStack
    import concourse.bass as bass
    import concourse.mybir as mybir

    dt = mybir.dt
    F32, BF16 = dt.float32, dt.bfloat16
    AF = mybir.ActivationFunctionType
    OP = mybir.AluOpType
    nc = bass.Bass()

    d_in = {}
    for name, shape, ty in [
        ("xtq", [DIM, NQ], BF16), ("xt", [DIM, N], BF16),
        ("wq", [DIM, INNER], BF16), ("srow", [1, INNER], BF16),
        ("wkv", [DIM, 2 * DIM_HEAD], BF16), ("nkvt", [128, NUM_NULL], BF16),
        ("wout", [INNER, DIM], BF16), ("ibf", [128, 128], BF16),
        ("onesbf", [1, 128], BF16), ("onesf", [1, 128], F32),
        ("ebias", [NPAIR, 128, EWP], BF16),
    ]:
        d_in[name] = nc.declare_dram_parameter(name, shape, ty, isOutput=False)
    out_d = nc.declare_dram_parameter("out", [DIM, NQ], F32, isOutput=True)

    ctx = ExitStack()
    sb = {}
    for name, shape, ty in [
        ("ibf", [128, 128], BF16), ("onesbf", [1, 128], BF16),
        ("onesf", [1, 128], F32), ("srow", [1, INNER], BF16),
        ("zb", [128, 1], F32), ("epsb", [128, 1], F32),
        ("onescol", [128, 1], BF16),
        ("wq", [128, 8 * INNER], BF16), ("wout", [128, 8 * DIM], BF16),
        ("wkv", [128, 8 * 128], BF16), ("xt", [128, 8 * N], BF16),
        ("xtq", [128, 8 * NQ], BF16), ("xsq", [128, 8 * NQ], BF16),
        ("lnrow", [1, 1024], F32),  # 0:256 negmu, 256:512 ex2, 512:768 var, 768:1024 rsq
        ("negmurs", [1, NQ], BF16),
        ("xst", [128, 8 * NQ], BF16), ("qtmp", [128, 2 * NQ], BF16),
        ("kv", [128, JPAD], BF16), ("vsb", [128, JT * 65], BF16),
        ("e0", [128, EWP], BF16), ("e1", [128, EWP], BF16),
        ("b0", [128, EWP], BF16), ("b1", [128, EWP], BF16),
        ("oT", [64, HEADS * NQ], BF16), ("rs_row", [65, 2 * 512], F32),
        ("rs64", [64, 64], F32), ("r64f", [64, 64], F32), ("r64b", [64, 64], BF16),
        ("recipflat", [1, HEADS * NQ], BF16),
        ("oTn", [128, 8 * NQ], BF16), ("oTn_lo", [64, 8 * NQ], BF16),
        ("outsb", [128, 2 * NQ], F32),
    ] + [(f"qh{h}", [64, 2 * NQ], BF16) for h in range(NPAIR)]:
        sb[name] = ctx.enter_context(nc.sbuf_tensor("sb_" + name, shape, ty))

    qh = [sb[f"qh{h}"] for h in range(NPAIR)]
    esb = [sb["e0"], sb["e1"]]
    bsb = [sb["b0"], sb["b1"]]

    # PSUM: early tensors freed before head-loop tensors are allocated.
    early = ExitStack()
    kvp = early.enter_context(nc.psum_tensor("kvp", [128, 1024], F32))
    qp = [early.enter_context(nc.psum_tensor(f"qp{i}", [128, NQ], F32))
          for i in range(2)]
    vp = [early.enter_context(nc.psum_tensor(f"vp{i}", [128, 64], BF16))
          for i in range(2)]
    stp2 = early.enter_context(nc.psum_tensor("stp2", [1, 512], F32))
    rbp = early.enter_context(nc.psum_tensor("rbp", [128, NQ], F32))
    early.close()
    simA = ctx.enter_context(nc.psum_tensor("simA", [128, 1536], F32))
    simB = ctx.enter_context(nc.psum_tensor("simB", [128, 1536], F32))
    opp2 = ctx.enter_context(nc.psum_tensor("opp2", [65, 1024], F32))

    # ------- planner -------
    plan = {"sync": [], "tensor": [], "vector": [], "scalar": [], "gpsimd": []}
    DSEMS = ("dk", "dxq", "dqx", "dsm", "dw",
             "db0", "db1", "dq", "dr", "dm", "dn", "do")
    cnt = {"p": 0, "v": 0, "s": 0, "g": 0, **{k: 0 for k in DSEMS}}
    SEM = {}
    ESEM = {"vector": "v", "gpsimd": "g", "scalar": "s"}

    def wait(eng, sem, thr):
        if thr > 0:
            plan[eng].append(lambda e, s=sem, t=thr: e.wait_ge(SEM[s], t))

    def dma(sem, out, in_, eng="sync"):
        cnt[sem] += 16
        plan[eng].append(
            lambda e, s=sem, o=out, i=in_: e.dma_start(out=o, in_=i)
            .then_inc(SEM[s], 16))
        return cnt[sem]

    def inc(eng, sem, fn):
        # DVE/ACT/Pool pipelines expose writes only after a drain; put the sem
        # update on the drain so consumers see committed data.
        cnt[sem] += 1
        if eng in ("vector", "scalar", "gpsimd"):
            plan[eng].append(lambda e, f=fn: f(e))
            plan[eng].append(lambda e, s=sem: e.drain().then_inc(SEM[s], 1))
        else:
            plan[eng].append(lambda e, f=fn, s=sem: f(e).then_inc(SEM[s], 1))
        return cnt[sem]

    def run(eng, fn):
        plan[eng].append(fn)

    def runD(eng, fn):
        # run + drain, for same-engine RAW chains without a semaphore inc
        plan[eng].append(fn)
        plan[eng].append(lambda e: e.drain())

    def mm_reuse(e, *args, **kwargs):
        # matmul whose stationary was loaded by the immediately preceding
        # matmul on PE: suppress this instruction's LDWEIGHTS.
        m = e.matmul(*args, **kwargs)
        m.ins.ldweights = False
        return m

    # ========== DMA issue order (sync queue = priority order) ==========
    def dma8(sem, nm, eng="sync"):
        return dma(sem, sb[nm][:].rearrange("p (c f) -> p c f", c=8),
                   d_in[nm][:].rearrange("(c p) f -> p c f", c=8), eng=eng)

    d_xq = dma8("dxq", "xtq")
    dma8("dqx", "wq")
    d_qx = cnt["dqx"]
    for nm in ("ibf", "onesbf", "onesf", "srow"):
        dma("dsm", sb[nm][:], d_in[nm][:])
    d_sm0 = dma("dsm", sb["kv"][:, 0:NUM_NULL], d_in["nkvt"][:])
    dma8("dk", "wkv")
    dma8("dk", "xt")
    d_kv = cnt["dk"]
    for p in range(2):
        dma(f"db{p}", bsb[p][:], d_in["ebias"][p])
    d_w = dma8("dw", "wout")  # after ebias: not needed until the tail

    # ========== VECTOR: memsets ==========
    run("vector", lambda e: e.memset(sb["zb"][:], 0.0))
    run("vector", lambda e: e.memset(sb["epsb"][:], EPS))
    run("vector", lambda e: e.memset(sb["onescol"][:], 1.0))
    run("vector", lambda e: e.memset(sb["vsb"][:], 1.0))
    v_memset = inc("vector", "v",
                   lambda e: e.memset(sb["kv"][:, NUM_NULL + N:JPAD], 0.0))

    # ========== LN stats (row layout, raw moments) ==========
    wait("scalar", "dxq", d_xq)
    wait("scalar", "v", v_memset)  # zb ready
    s_xsq = inc("scalar", "s", lambda e: e.activation(
        out=sb["xsq"][:], in_=sb["xtq"][:], func=AF.Square, bias=sb["zb"][:]))

    # tensor: musum (cols 0:256) and sqsum (cols 256:512) into stp2;
    # all 16 matmuls share the onescol stationary (one LDWEIGHTS).
    wait("tensor", "dxq", d_xq)
    wait("tensor", "v", v_memset)  # onescol ready
    for ct in range(8):
        def fn(e, ct=ct):
            f = e.matmul if ct == 0 else (lambda *a, **k: mm_reuse(e, *a, **k))
            return f(stp2[0:1, 0:NQ], sb["onescol"][:],
                     sb["xtq"][:, ct * NQ:(ct + 1) * NQ],
                     start=(ct == 0), stop=(ct == 7))
        if ct == 7:
            p_mu = inc("tensor", "p", fn)
        else:
            run("tensor", fn)

    wait("tensor", "s", s_xsq)
    for ct in range(8):
        fn = lambda e, ct=ct: mm_reuse(
            e, stp2[0:1, NQ:2 * NQ], sb["onescol"][:],
            sb["xsq"][:, ct * NQ:(ct + 1) * NQ],
            start=(ct == 0), stop=(ct == 7))
        if ct == 7:
            p_sq = inc("tensor", "p", fn)
        else:
            run("tensor", fn)

    # vector rowops (RAW chains -> drained)
    neg_mu = sb["lnrow"][0:1, 0:256]
    ex2 = sb["lnrow"][0:1, 256:512]
    var_r = sb["lnrow"][0:1, 512:768]
    rsq_r = sb["lnrow"][0:1, 768:1024]
    wait("vector", "p", p_mu)
    runD("vector", lambda e: e.tensor_scalar_mul(
        out=neg_mu, in0=stp2[0:1, 0:NQ], scalar1=-1.0 / DIM))
    wait("vector", "p", p_sq)
    runD("vector", lambda e: e.tensor_scalar_mul(
        out=ex2, in0=stp2[0:1, NQ:2 * NQ], scalar1=1.0 / DIM))
    runD("vector", lambda e: e.tensor_tensor(
        out=var_r, in0=neg_mu, in1=neg_mu, op=OP.mult))
    v_var = inc("vector", "v", lambda e: e.tensor_tensor(
        out=var_r, in0=ex2, in1=var_r, op=OP.subtract))
    # scalar: rsq = exp(-0.5 * ln(var + eps))
    wait("scalar", "v", v_var)
    runD("scalar", lambda e: e.activation(
        out=var_r, in_=var_r, func=AF.Ln, scale=1.0, bias=sb["epsb"][0:1, :]))
    s_rsq = inc("scalar", "s", lambda e: e.activation(
        out=rsq_r, in_=var_r, func=AF.Exp, scale=-0.5, bias=sb["zb"][0:1, :]))
    wait("vector", "s", s_rsq)
    v_nmr = inc("vector", "v", lambda e: e.tensor_tensor(
        out=sb["negmurs"][:], in0=neg_mu, in1=rsq_r, op=OP.mult))

    # tensor: rsqb broadcast [128, 256] (f32 matmul)
    wait("tensor", "dsm", d_sm0)  # onesf (+ibf/srow/nkvt) loaded
    wait("tensor", "s", s_rsq)
    p_rsqb = inc("tensor", "p", lambda e: e.matmul(
        rbp[:], sb["onesf"][0:1, :], rsq_r, start=True, stop=True))

    # vector: xst = xtq * rsqb
    wait("vector", "p", p_rsqb)
    for ct in range(8):
        fn = lambda e, ct=ct: e.tensor_tensor(
            out=sb["xst"][:, ct * NQ:(ct + 1) * NQ],
            in0=sb["xtq"][:, ct * NQ:(ct + 1) * NQ], in1=rbp[:], op=OP.mult)
        if ct == 7:
            v_xst = inc("vector", "v", fn)
        else:
            run("vector", fn)

    # ========== TENSOR: q projection (qp double-buffered) ==========
    wait("tensor", "v", v_xst)
    wait("tensor", "dqx", d_qx)
    wait("tensor", "v", v_nmr)
    p_q = [0] * 8
    v_qtmp = [0] * 8
    for dtile in range(8):
        pb = qp[dtile % 2]
        if dtile >= 2:
            wait("tensor", "v", v_qtmp[dtile - 2])
        for ct in range(8):
            run("tensor", lambda e, pb=pb, dtile=dtile, ct=ct: e.matmul(
                pb[:],
                sb["wq"][:, ct * INNER + dtile * 128:ct * INNER + (dtile + 1) * 128],
                sb["xst"][:, ct * NQ:(ct + 1) * NQ],
                start=(ct == 0), stop=False))
        p_q[dtile] = inc("tensor", "p", lambda e, pb=pb, dtile=dtile: e.matmul(
            pb[:], sb["srow"][0:1, dtile * 128:(dtile + 1) * 128],
            sb["negmurs"][:], start=False, stop=True))
        wait("vector", "p", p_q[dtile])
        run("vector", lambda e, pb=pb, dtile=dtile: e.tensor_copy(
            qh[dtile][0:64, :].rearrange("a (b h c) -> a b h c", b=2, h=2)[:, :, 0, :],
            pb[0:64, :].rearrange("a (b c) -> a b c", b=2)))
        slot = dtile % 2
        if dtile >= 2:
            wait("vector", "dq", 16 * dtile)  # qtmp slot reuse (all issued)
        v_qtmp[dtile] = inc("vector", "v", lambda e, pb=pb, slot=slot:
                            e.tensor_copy(
                                sb["qtmp"][64:128, slot * NQ:(slot + 1) * NQ],
                                pb[64:128, :]))
        wait("sync", "v", v_qtmp[dtile])
        dma("dq",
            qh[dtile][0:64, :].rearrange("a (b h c) -> a b h c", b=2, h=2)[:, :, 1, :],
            sb["qtmp"][64:128, slot * NQ:(slot + 1) * NQ]
            .rearrange("a (b c) -> a b c", b=2))

    # ========== TENSOR: kv matmuls, ct-outer (one wkv load per ct) ==========
    wait("tensor", "dk", d_kv)
    s_kvevac = [0] * 4
    p_kvh = [0, 0]
    for half in range(2):
        if half == 1:
            wait("tensor", "s", s_kvevac[1])  # kvp reuse
        for ct in range(8):
            for chh in range(2):
                ch = half * 2 + chh
                def fn(e, ct=ct, ch=ch, chh=chh):
                    f = e.matmul if chh == 0 else (
                        lambda *a, **k: mm_reuse(e, *a, **k))
                    return f(
                        kvp[:, chh * 512:(chh + 1) * 512],
                        sb["wkv"][:, ct * 128:(ct + 1) * 128],
                        sb["xt"][:, ct * N + ch * 512:ct * N + (ch + 1) * 512],
                        start=(ct == 0), stop=(ct == 7))
                if ct == 7 and chh == 1:
                    p_kvh[half] = inc("tensor", "p", fn)
                else:
                    run("tensor", fn)
        wait("scalar", "p", p_kvh[half])
        for chh in range(2):
            ch = half * 2 + chh
            s_kvevac[ch] = inc("scalar", "s", lambda e, ch=ch, chh=chh:
                               e.activation(
                                   out=sb["kv"][:, NUM_NULL + ch * 512:
                                                NUM_NULL + (ch + 1) * 512],
                                   in_=kvp[:, chh * 512:(chh + 1) * 512],
                                   func=AF.Copy))

    # ========== TENSOR: v transposes (vp double-buffered) ==========
    p_vt = [0] * JT
    v_vcopy = [0] * JT
    for jt in range(JT):
        pb = vp[jt % 2]
        ch_hi = min(3, ((jt + 1) * 128 - 1 - NUM_NULL) // 512)
        wait("tensor", "s", s_kvevac[ch_hi])
        if jt == JT - 1:
            wait("tensor", "v", v_memset)
        if jt >= 2:
            wait("tensor", "v", v_vcopy[jt - 2])
        p_vt[jt] = inc("tensor", "p", lambda e, pb=pb, jt=jt: e.transpose(
            pb[:], sb["kv"][64:128, jt * 128:(jt + 1) * 128],
            sb["ibf"][64:128, 64:128]))
        wait("vector", "p", p_vt[jt])
        v_vcopy[jt] = inc("vector", "v", lambda e, pb=pb, jt=jt: e.tensor_copy(
            sb["vsb"][:, jt * 65:jt * 65 + 64], pb[:]))
    v_vsb = v_vcopy[JT - 1]

    # ========== PAIR LOOP: sims per pair, o-matmuls joint per pair-pair ====
    v_pre_heads = cnt["v"]
    s_exp = [[0] * NCH for _ in range(NPAIR)]
    e_mult = [[None] * NCH for _ in range(NPAIR)]  # (sem_name, count)
    p_simc = [[0] * NCH for _ in range(NPAIR)]
    p_odone = [0] * (NPAIR // 2)
    v_oevac = [0] * (NPAIR // 2)

    wait("tensor", "s", s_kvevac[3])
    wait("tensor", "v", v_pre_heads)  # early-psum reuse guard
    wait("tensor", "dq", 16 * 8)      # all odd-half q DMAs done

    def emit_pair_sims(p):
        """sims + exp + E-mult for pair p (5 chunks)."""
        eh = esb[p % 2]
        bh = bsb[p % 2]
        qpair = qh[p][0:64, :]
        qsolo = qh[p][0:64, NQ:2 * NQ]
        for ck in range(NCH):
            ps = simA if ck in (0, 2, 4) else simB
            jts = [jt for jt in range(JT) if CH_OF_JT[jt] == ck]
            # psum region reuse: wait on exp of the previous occupant
            if ck in (0, 1):
                if p >= 1:
                    wait("tensor", "s", s_exp[p - 1][{0: 4, 1: 3}[ck]])
            else:
                wait("tensor", "s", s_exp[p][ck - 2])
            for jt in jts:
                w = 512 if jt < SH_JT else 256
                rhs = qpair if jt < SH_JT else qsolo
                fn = lambda e, ps=ps, jt=jt, r=rhs, w=w: e.matmul(
                    ps[:, _ccol(jt):_ccol(jt) + w],
                    sb["kv"][0:64, jt * 128:(jt + 1) * 128], r,
                    start=True, stop=True)
                if jt == jts[-1]:
                    p_simc[p][ck] = inc("tensor", "p", fn)
                else:
                    run("tensor", fn)
            # SCALAR: exp for this chunk
            wait("scalar", "p", p_simc[p][ck])
            if ck == 0 and p >= 2:
                wait("scalar", "p", p_odone[p // 2 - 1])  # E slot reuse
            ln = CH_LEN[ck]
            s_exp[p][ck] = inc("scalar", "s", lambda e, ps=ps, ck=ck, ln=ln,
                               eh=eh: e.activation(
                                   out=eh[:, CH_ECOL[ck]:CH_ECOL[ck] + ln],
                                   in_=ps[:, 0:ln], func=AF.Exp, bias=sb["zb"][:]))
            # DVE/Pool: E *= exp(bias) in place (bf16, all-SBUF)
            eng = CH_ENG[ck]
            sem = ESEM[eng]
            if ck in (0, 1):  # first chunk on each engine: ebias slot loaded
                wait(eng, f"db{p % 2}", 16 * (p // 2 + 1))
            wait(eng, "s", s_exp[p][ck])
            n = inc(eng, sem, lambda e, ck=ck, ln=ln, eh=eh, bh=bh:
                    e.tensor_tensor(
                        out=eh[:, CH_ECOL[ck]:CH_ECOL[ck] + ln],
                        in0=eh[:, CH_ECOL[ck]:CH_ECOL[ck] + ln],
                        in1=bh[:, CH_ECOL[ck]:CH_ECOL[ck] + ln],
                        op=OP.mult))
            e_mult[p][ck] = (sem, n)

    for pp in range(NPAIR // 2):
        pa, pb_ = 2 * pp, 2 * pp + 1
        emit_pair_sims(pa)
        emit_pair_sims(pb_)

        # TENSOR: joint o-matmuls for pairs (pa, pb_): per jt one V-tile
        # load serves both pairs (second matmul reuses the stationary).
        if pp == 0:
            wait("tensor", "v", v_vsb)
        if pp >= 1:
            wait("tensor", "v", v_oevac[pp - 1])  # opp2 reuse
        ea, eb_ = esb[0], esb[1]
        for jt in range(JT):
            ck = CH_OF_JT[jt]
            if jt == 0 or _ccol(jt) == 0:
                for pq in (pa, pb_):
                    sem, n = e_mult[pq][ck]
                    wait("tensor", sem, n)
            w, eoff = (512, 0) if jt < SH_JT else (256, 256)
            fn_a = lambda e, jt=jt, w=w, eoff=eoff: e.matmul(
                opp2[0:65, eoff:eoff + w],
                sb["vsb"][:, jt * 65:jt * 65 + 65],
                ea[:, _ecol(jt):_ecol(jt) + w],
                start=(jt == 0), stop=(jt == JT - 1))
            fn_b = lambda e, jt=jt, w=w, eoff=eoff: mm_reuse(
                e, opp2[0:65, 512 + eoff:512 + eoff + w],
                sb["vsb"][:, jt * 65:jt * 65 + 65],
                eb_[:, _ecol(jt):_ecol(jt) + w],
                start=(jt == 0), stop=(jt == JT - 1))
            run("tensor", fn_a)
            if jt == JT - 1:
                p_odone[pp] = inc("tensor", "p", fn_b)
            else:
                run("tensor", fn_b)

        # VECTOR: evacuate o rows + rowsum rows for both pairs
        wait("vector", "p", p_odone[pp])
        if pp >= 1:
            wait("vector", "dr", 64 * pp)  # rs_row slots reuse (all issued)
        for si, pq in enumerate((pa, pb_)):
            run("vector", lambda e, si=si, pq=pq: e.tensor_copy(
                sb["oT"][0:64, pq * 512:(pq + 1) * 512]
                .rearrange("a (h b c) -> a h b c", h=2, b=2),
                opp2[0:64, si * 512:(si + 1) * 512]
                .rearrange("a (b h c) -> a h b c", b=2, h=2)))
        for si, pq in enumerate((pa, pb_)):
            fn = lambda e, si=si: e.tensor_copy(
                sb["rs_row"][64:65, si * 512:(si + 1) * 512]
                .rearrange("a (h b c) -> a h b c", h=2, b=2),
                opp2[64:65, si * 512:(si + 1) * 512]
                .rearrange("a (b h c) -> a h b c", b=2, h=2))
            if si == 1:
                v_oevac[pp] = inc("vector", "v", fn)
            else:
                run("vector", fn)
        # SYNC: scatter rowsums into rs64. Layout: partition q = h*32 + p*4
        # + bc//64, col = bc%64 -- partition-major iteration of rs64 is then
        # exactly the even/odd-head-permuted recipflat order.
        wait("sync", "v", v_oevac[pp])
        for si, pq in enumerate((pa, pb_)):
            for h in range(2):
                dma("dr", sb["rs64"][h * 32 + pq * 4:h * 32 + pq * 4 + 4, :],
                    sb["rs_row"][64:65,
                                 si * 512 + h * 256:si * 512 + (h + 1) * 256])

        # SYNC: ebias prefetch for pairs pa+2, pb_+2 (slot free once both
        # engines' E-mults for the current occupant finish)
        for pq in (pa, pb_):
            if pq + 2 < NPAIR:
                for s in ("v", "g"):
                    ent = [e_mult[pq][c] for c in range(NCH)
                           if e_mult[pq][c][0] == s]
                    if ent:
                        wait("sync", s, ent[-1][1])
                dma(f"db{pq % 2}", bsb[pq % 2][:], d_in["ebias"][pq + 2])

    # ========== tail: reciprocal + normalize + output projection ==========
    wait("vector", "dr", cnt["dr"])
    runD("vector", lambda e: e.reciprocal(out=sb["r64f"][:], in_=sb["rs64"][:]))
    v_recip = inc("vector", "v", lambda e: e.tensor_copy(
        sb["r64b"][:], sb["r64f"][:]))
    # direct SBUF->SBUF flatten: partition-major r64b == permuted order
    wait("sync", "v", v_recip)
    d_m = dma("dm", sb["recipflat"][0:1, :], sb["r64b"][:])

    # PE: broadcast matmuls R = ones64 x recipflat (one LDWEIGHTS total)
    wait("tensor", "dm", d_m)
    wait("tensor", "s", s_exp[NPAIR - 1][NCH - 1])  # simA/simB free
    wait("tensor", "v", v_oevac[NPAIR // 2 - 1])    # opp2 free
    p_bc = [0] * 3
    # R_even (heads 0,2,..14) -> simB (3 x 512) + opp2[0:64, 0:512]
    # R_odd  (heads 1,3,..15) -> simA (3 x 512) + opp2[0:64, 512:1024]
    bc_dsts = ([(simB, i * 512, i * 512) for i in range(3)]
               + [(opp2, 0, 1536)]
               + [(simA, i * 512, 2048 + i * 512) for i in range(3)]
               + [(opp2, 512, 3584)])
    for i, (ps, poff, roff) in enumerate(bc_dsts):
        fn = lambda e, i=i, ps=ps, poff=poff, roff=roff: (
            e.matmul if i == 0 else (lambda *a, **k: mm_reuse(e, *a, **k)))(
            ps[0:64, poff:poff + 512], sb["onesbf"][0:1, 0:64],
            sb["recipflat"][0:1, roff:roff + 512], start=True, stop=True)
        if i in (3, 6, 7):
            p_bc[{3: 0, 6: 1, 7: 2}[i]] = inc("tensor", "p", fn)
        else:
            run("tensor", fn)

    # VECTOR: oTn = oT * R  (even heads -> oTn[0:64], odd -> oTn_lo)
    oT_hp = sb["oT"][0:64, :].rearrange("a (p k c) -> a p k c", p=8, k=2)
    wait("vector", "p", p_bc[0])
    v_n0 = inc("vector", "v", lambda e: e.tensor_tensor(
        out=sb["oTn"][0:64, 0:1536].rearrange("a (p c) -> a p c", p=6),
        in0=oT_hp[:, 0:6, 0, :],
        in1=simB[0:64, 0:1536].rearrange("a (p c) -> a p c", p=6),
        op=OP.mult))
    run("vector", lambda e: e.tensor_tensor(
        out=sb["oTn"][0:64, 1536:2048].rearrange("a (p c) -> a p c", p=2),
        in0=oT_hp[:, 6:8, 0, :],
        in1=opp2[0:64, 0:512].rearrange("a (p c) -> a p c", p=2),
        op=OP.mult))
    wait("vector", "p", p_bc[1])
    run("vector", lambda e: e.tensor_tensor(
        out=sb["oTn_lo"][0:64, 0:1536].rearrange("a (p c) -> a p c", p=6),
        in0=oT_hp[:, 0:6, 1, :],
        in1=simA[0:64, 0:1536].rearrange("a (p c) -> a p c", p=6),
        op=OP.mult))
    wait("vector", "p", p_bc[2])
    v_n2 = inc("vector", "v", lambda e: e.tensor_tensor(
        out=sb["oTn_lo"][0:64, 1536:2048].rearrange("a (p c) -> a p c", p=2),
        in0=oT_hp[:, 6:8, 1, :],
        in1=opp2[0:64, 512:1024].rearrange("a (p c) -> a p c", p=2),
        op=OP.mult))
    wait("sync", "v", v_n2)
    d_n = dma("dn", sb["oTn"][64:128, :], sb["oTn_lo"][0:64, :])

    # ========== output projection (accumulators alternate in simB banks) ====
    wait("tensor", "v", v_n0)
    wait("tensor", "dn", d_n)
    wait("tensor", "dw", d_w)
    s_outevac = [0] * 8
    for et in range(8):
        reg = simB[:, 0:NQ] if et % 2 == 0 else simB[:, 512:512 + NQ]
        if et >= 2:
            wait("tensor", "s", s_outevac[et - 2])
        p_wout = 0
        for hdt in range(8):
            fn = lambda e, et=et, hdt=hdt, reg=reg: e.matmul(
                reg, sb["wout"][:, hdt * DIM + et * 128:hdt * DIM + (et + 1) * 128],
                sb["oTn"][:, hdt * NQ:(hdt + 1) * NQ],
                start=(hdt == 0), stop=(hdt == 7))
            if hdt == 7:
                p_wout = inc("tensor", "p", fn)
            else:
                run("tensor", fn)
        wait("scalar", "p", p_wout)
        oslot = et % 2
        if et >= 2:
            wait("scalar", "do", 16 * et)  # outsb slot reuse (all issued)
        s_outevac[et] = inc("scalar", "s", lambda e, oslot=oslot, reg=reg:
                            e.activation(
                                out=sb["outsb"][:, oslot * NQ:(oslot + 1) * NQ],
                                in_=reg, func=AF.Copy))
        wait("sync", "s", s_outevac[et])
        dma("do", out_d[et * 128:(et + 1) * 128, :],
            sb["outsb"][:, oslot * NQ:(oslot + 1) * NQ])

    # ========== emit ==========
    from contextlib import ExitStack as _ES
    semctx = _ES()
    for k in ("p", "v", "s", "g") + DSEMS:
        SEM[k] = semctx.enter_context(nc.semaphore(f"sem_{k}"))
    with semctx:
        with nc.Block() as block:
            @block.sync
            def _(e):
                for fn in plan["sync"]:
                    fn(e)

            @block.tensor
            def _(e):
                for fn in plan["tensor"]:
                    fn(e)

            @block.vector
            def _(e):
                for fn in plan["vector"]:
                    fn(e)

            @block.scalar
            def _(e):
                for fn in plan["scalar"]:
                    fn(e)

            @block.gpsimd
            def _(e):
                for fn in plan["gpsimd"]:
                    fn(e)
    ctx.close()
    return nc


def _prep_inputs(x, attn_bias, Wq, Wkv, null_kv, Wout, gamma, mask):
    from ml_dtypes import bfloat16
    x = np.asarray(x, np.float32)[0]            # [N, DIM]
    attn_bias = np.asarray(attn_bias, np.float32)[0]  # [H, N, N]
    Wq = np.asarray(Wq, np.float32)
    Wkv = np.asarray(Wkv, np.float32)
    null_kv = np.asarray(null_kv, np.float32)
    Wout = np.asarray(Wout, np.float32)
    gamma = np.asarray(gamma, np.float32)
    mask = np.asarray(mask, bool)[0]            # [N]

    scale = DIM_HEAD ** -0.5
    wq_eff = (gamma[:, None] * Wq * scale).astype(np.float32)
    srow = wq_eff.sum(axis=0, keepdims=True)
    xt = np.ascontiguousarray(x.T)
    nkvt = np.zeros((128, NUM_NULL), np.float32)
    nkvt[0:DIM_HEAD, :] = null_kv[0].T
    nkvt[64:64 + DIM_HEAD, :] = null_kv[1].T
    I128 = np.eye(128, dtype=np.float32)
    ones = np.ones((1, 128), np.float32)

    jpad = np.arange(JPAD)
    jvalid = np.zeros(JPAD, bool)
    jvalid[:NUM_NULL] = True
    jvalid[NUM_NULL:NUM_NULL + N] = mask
    key_of_j = jpad - NUM_NULL

    in_maps = []
    idx_all = []
    for c in range(NCORES):
        idx = np.concatenate([np.arange(c, 1024, 8), np.arange(1024 + c, 2048, 8)])
        idx_all.append(idx)
        allow = jvalid[None, :] & (key_of_j[None, :] <= idx[:, None])  # [NQ, JPAD]
        allow[:, :NUM_NULL] = True
        ab = np.zeros((HEADS, JPAD, NQ), np.float32)
        ab[:, NUM_NULL:NUM_NULL + N, :] = attn_bias[:, idx, :].transpose(0, 2, 1)
        bt = np.exp(np.where(allow.T[None], ab, MASK_VAL))  # exp(bias), masked->0
        pk = np.empty((HEADS // 2, 128, EWP), np.float32)
        for jt in range(SH_JT):
            c0 = jt * 512
            tile = bt[:, jt * 128:(jt + 1) * 128, :]        # [H, 128, 256]
            pk[:, :, c0:c0 + 128] = tile[0::2, :, 0:128]         # h0 b0
            pk[:, :, c0 + 128:c0 + 256] = tile[1::2, :, 0:128]   # h1 b0
            pk[:, :, c0 + 256:c0 + 384] = tile[0::2, :, 128:256]  # h0 b1
            pk[:, :, c0 + 384:c0 + 512] = tile[1::2, :, 128:256]  # h1 b1
        for jt in range(SH_JT, JT):
            c0 = SH_JT * 512 + (jt - SH_JT) * 256
            tile = bt[:, jt * 128:(jt + 1) * 128, 128:256]  # [H, 128, 128]
            pk[:, :, c0:c0 + 128] = tile[0::2]
            pk[:, :, c0 + 128:c0 + 256] = tile[1::2]
        in_maps.append({
            "xtq": np.ascontiguousarray(xt[:, idx]).astype(bfloat16),
            "xt": xt.astype(bfloat16),
            "wq": wq_eff.astype(bfloat16),
            "srow": srow.astype(bfloat16),
            "wkv": Wkv.astype(bfloat16),
            "nkvt": nkvt.astype(bfloat16),
            "wout": Wout.astype(bfloat16),
            "ibf": I128.astype(bfloat16),
            "onesbf": ones.astype(bfloat16),
            "onesf": ones,
            "ebias": pk.astype(bfloat16),
        })
    return in_maps, idx_all


def _run(inputs, trace=False):
    from concourse.bass_utils import run_bass_kernel_spmd
    if "nc" not in _CACHE:
        _CACHE["nc"] = _build_graph()
    nc = _CACHE["nc"]
    in_maps, idx_all = _prep_inputs(**inputs)
    res = run_bass_kernel_spmd(nc, in_maps, list(range(NCORES)), trace=trace)
    out = np.zeros((B, N, DIM), np.float32)
    for c in range(NCORES):
        out[0, idx_all[c], :] = res.results[c]["out"].T
    return out, res


def kernel(**inputs):
    out, _ = _run(inputs, trace=False)
    return out
